# revision 11
# baseline (speedup 1.0000x reference)
"""Trainium2 Bass kernel for nn_Attention (B=4, Nq=Nk=1024, D=512, H=8).

Sharding: 8 cores = 4 batches x 2 head-groups (4 heads each).
Core c handles batch b = c // 2, heads [hg*4, hg*4+4) with hg = c % 2.

Per-core device program (all heavy math on device):
  qT = (x_q[b] @ w_q[:, hg])^T          [256, 1024]   (f32r matmuls)
  kT = (x[b]  @ w_k[:, hg])^T           [256, 1024]
  v  =  x[b]  @ w_v[:, hg]              [1024, 256]   (stored bf16)
  per head h (4):
    A: logits[q,k] = qT_h^T kT_h; exp (ACT, scale=0.125 fused, accum row
       sums); normalize in-place (gpsimd, per-partition recip); DMA -> attn
    B: logitsT[k,q] = kT_h^T qT_h; expT = exp (ACT, bf16)
    AV: out_u[q,64] = sum_kj expT_kj^T v_kj (bf16 matmuls);
        scale by recip (DVE, fused into PSUM->SBUF copy)
  transpose out [1024,256] -> outT (PE transposes), proj = outT^T @ w_p
  partial out -> DRAM (host adds the two head-group partials + bias).

Host side: slice/transpose inputs (numpy), run SPMD on 8 cores, concat
attn shards, add the two per-batch partials + b_proj.
"""

import sys

import numpy as np

for _p in ("/opt/trn_rl_repo",):
    if _p not in sys.path:
        sys.path.insert(0, _p)

# Problem constants (hardcoded per contest rules).
B, NQ, NK = 4, 1024, 1024
D = 512          # DIM_Q = DIM_K = OUT_DIM
H = 8
HD = 64          # head dim
SCALE = HD ** -0.5
HPC = 4          # heads per core
DH = HPC * HD    # 256: per-core slice of head dims
P = 128
KO = D // P      # 4 contraction chunks for the input projections
NCORES = 8

_NC_CACHE = {}


def build_nc():
    """Build the (single) SPMD Bass program. Same program on all 8 cores."""
    from contextlib import ExitStack

    import concourse.mybir as mybir
    import concourse.tile as tile
    from concourse import bacc
    from concourse.masks import make_identity

    FP = mybir.dt.float32
    BF = mybir.dt.bfloat16
    FR = mybir.dt.float32r
    Exp = mybir.ActivationFunctionType.Exp

    nc = bacc.Bacc("TRN2")
    xqT = nc.declare_dram_parameter("xqT", [D, NQ], FR, isOutput=False)
    xT = nc.declare_dram_parameter("xT", [D, NK], FR, isOutput=False)
    wq = nc.declare_dram_parameter("wq", [D, DH], FR, isOutput=False)
    wk = nc.declare_dram_parameter("wk", [D, DH], FR, isOutput=False)
    wv = nc.declare_dram_parameter("wv", [D, DH], FR, isOutput=False)
    wp = nc.declare_dram_parameter("wp", [DH, D], BF, isOutput=False)
    attn_o = nc.declare_dram_parameter("attn_o", [HPC, NQ, NK], FP, isOutput=True)
    out_o = nc.declare_dram_parameter("out_o", [NQ, D], FP, isOutput=True)

    with tile.TileContext(nc) as tc:
        with ExitStack() as ctx:
            consts = ctx.enter_context(tc.tile_pool(name="consts", bufs=1))
            persist = ctx.enter_context(tc.tile_pool(name="persist", bufs=1))
            expT_pool = ctx.enter_context(tc.tile_pool(name="expT", bufs=2))
            exA_pool = ctx.enter_context(tc.tile_pool(name="exA", bufs=10))
            outp = ctx.enter_context(tc.tile_pool(name="outp", bufs=2))
            ps_mm = ctx.enter_context(tc.tile_pool(name="ps_mm", bufs=1, space="PSUM"))
            ps_lg = ctx.enter_context(tc.tile_pool(name="ps_lg", bufs=3, space="PSUM"))
            ps_av = ctx.enter_context(tc.tile_pool(name="ps_av", bufs=1, space="PSUM"))

            # ---- load inputs ----
            xqT_sb = persist.tile([P, KO, NQ], FR)
            nc.sync.dma_start(xqT_sb[:], xqT[:].rearrange("(ko p) q -> p ko q", p=P))
            xT_sb = persist.tile([P, KO, NK], FR)
            nc.sync.dma_start(xT_sb[:], xT[:].rearrange("(ko p) q -> p ko q", p=P))
            wq_sb = persist.tile([P, KO, DH], FR)
            nc.sync.dma_start(wq_sb[:], wq[:].rearrange("(ko p) m -> p ko m", p=P))
            wk_sb = persist.tile([P, KO, DH], FR)
            nc.sync.dma_start(wk_sb[:], wk[:].rearrange("(ko p) m -> p ko m", p=P))
            wv_sb = persist.tile([P, KO, DH], FR)
            nc.sync.dma_start(wv_sb[:], wv[:].rearrange("(ko p) m -> p ko m", p=P))
            wp_sb = persist.tile([P, 2, D], BF)
            nc.sync.dma_start(wp_sb[:], wp[:].rearrange("(j p) n -> p j n", p=P))
            ident = consts.tile([P, P], BF)
            make_identity(nc, ident[:])

            # ---- projections: qT, kT  ([dh-part(2x128), seq]) ----
            qT_sb = persist.tile([P, 2, NQ], BF)
            kT_sb = persist.tile([P, 2, NK], BF)
            for w_sb, x_sb, dst in ((wq_sb, xqT_sb, qT_sb), (wk_sb, xT_sb, kT_sb)):
                for j in range(2):
                    for qc in range(2):
                        ps = ps_mm.tile([P, 512], FP, tag="mm")
                        for ko in range(KO):
                            nc.tensor.matmul(
                                ps[:],
                                w_sb[:, ko, j * P:(j + 1) * P],
                                x_sb[:, ko, qc * 512:(qc + 1) * 512],
                                start=(ko == 0),
                                stop=(ko == KO - 1),
                            )
                        nc.vector.tensor_copy(dst[:, j, qc * 512:(qc + 1) * 512], ps[:])

            # ---- v in natural layout [k-part, kj, head, hd], bf16 ----
            v_sb = persist.tile([P, 8, HPC, HD], BF)
            for kj in range(8):
                ps = ps_mm.tile([P, DH], FP, tag="mm")
                for ko in range(KO):
                    nc.tensor.matmul(
                        ps[:],
                        xT_sb[:, ko, kj * P:(kj + 1) * P],
                        wv_sb[:, ko, :],
                        start=(ko == 0),
                        stop=(ko == KO - 1),
                    )
                nc.vector.tensor_copy(
                    v_sb[:, kj].rearrange("p h d -> p (h d)"), ps[:]
                )

            # ---- per-head attention ----
            sums = consts.tile([P, HPC * 8], FP)   # row sums per (head, q-tile)
            rec = consts.tile([P, HPC * 8], FP)    # reciprocals
            out_n = persist.tile([P, 8, DH], BF)   # normalized attn @ v, [q, dh]

            for h in range(HPC):
                j, p0 = h // 2, (h % 2) * 64
                qT_h = qT_sb[p0:p0 + 64, j]        # [64, NQ]
                kT_h = kT_sb[p0:p0 + 64, j]        # [64, NK]

                # --- A: logits [q, k] -> exp (+ row sums) ---
                exa_tiles = []
                for mi in range(8):
                    ps = ps_lg.tile([P, NK], FP, tag="lg")
                    for kc in range(2):
                        nc.tensor.matmul(
                            ps[:, kc * 512:(kc + 1) * 512],
                            qT_h[:, mi * P:(mi + 1) * P],
                            kT_h[:, kc * 512:(kc + 1) * 512],
                            start=True,
                            stop=True,
                        )
                    exa = exA_pool.tile([P, NK], FP, tag="exA")
                    si = h * 8 + mi
                    nc.scalar.activation(
                        exa[:], ps[:], Exp, scale=SCALE,
                        accum_out=sums[:, si:si + 1],
                    )
                    exa_tiles.append(exa)

                nc.vector.reciprocal(rec[:, h * 8:(h + 1) * 8], sums[:, h * 8:(h + 1) * 8])

                # --- B: logitsT [k, q] -> expT (bf16) ---
                expT = expT_pool.tile([P, 8, NQ], BF, tag="expT")
                for kj in range(8):
                    ps = ps_lg.tile([P, NQ], FP, tag="lg")
                    for qc in range(2):
                        nc.tensor.matmul(
                            ps[:, qc * 512:(qc + 1) * 512],
                            kT_h[:, kj * P:(kj + 1) * P],
                            qT_h[:, qc * 512:(qc + 1) * 512],
                            start=True,
                            stop=True,
                        )
                    nc.scalar.activation(expT[:, kj], ps[:], Exp, scale=SCALE)

                # --- normalize attn in place + store ---
                for mi in range(8):
                    si = h * 8 + mi
                    nc.vector.tensor_scalar_mul(
                        exa_tiles[mi][:], exa_tiles[mi][:], rec[:, si:si + 1]
                    )
                    nc.sync.dma_start(
                        attn_o[h, mi * P:(mi + 1) * P, :], exa_tiles[mi][:]
                    )

                # --- AV: out_u[q, 64] = sum_kj expT_kj^T v_kj; scale by rec ---
                for mi in range(8):
                    psv = ps_av.tile([P, HD], FP, tag="av")
                    for kj in range(8):
                        nc.tensor.matmul(
                            psv[:],
                            expT[:, kj, mi * P:(mi + 1) * P],
                            v_sb[:, kj, h],
                            start=(kj == 0),
                            stop=(kj == 7),
                        )
                    si = h * 8 + mi
                    nc.vector.tensor_scalar_mul(
                        out_n[:, mi, h * HD:(h + 1) * HD], psv[:], rec[:, si:si + 1]
                    )

            # ---- transpose out [q, dh] -> outT [dh, q] ----
            outT = persist.tile([P, 2, NQ], BF)
            for mi in range(8):
                for j in range(2):
                    pst = ps_av.tile([P, P], BF, tag="av")
                    nc.tensor.transpose(pst[:], out_n[:, mi, j * P:(j + 1) * P], ident[:])
                    nc.vector.tensor_copy(outT[:, j, mi * P:(mi + 1) * P], pst[:])

            # ---- output projection ----
            for mi in range(8):
                ps = ps_mm.tile([P, D], FP, tag="mm")
                for j in range(2):
                    nc.tensor.matmul(
                        ps[:],
                        outT[:, j, mi * P:(mi + 1) * P],
                        wp_sb[:, j, :],
                        start=(j == 0),
                        stop=(j == 1),
                    )
                of = outp.tile([P, D], FP, tag="of")
                nc.vector.tensor_copy(of[:], ps[:])
                nc.sync.dma_start(out_o[mi * P:(mi + 1) * P, :], of[:])

    nc.compile()
    return nc


def get_nc():
    if "nc" not in _NC_CACHE:
        _NC_CACHE["nc"] = build_nc()
    return _NC_CACHE["nc"]


def make_in_maps(x, x_q, w_q, w_kv):
    """Shard full inputs into 8 per-core input maps (host-side numpy)."""
    x = np.asarray(x, dtype=np.float32)
    x_q = np.asarray(x_q, dtype=np.float32)
    w_q = np.asarray(w_q, dtype=np.float32)
    w_kv = np.asarray(w_kv, dtype=np.float32)
    in_maps = []
    for c in range(NCORES):
        b, hg = c // 2, c % 2
        sl = slice(hg * DH, (hg + 1) * DH)
        in_maps.append({
            "xqT": np.ascontiguousarray(x_q[b].T),
            "xT": np.ascontiguousarray(x[b].T),
            "wq": np.ascontiguousarray(w_q[:, sl]),
            "wk": np.ascontiguousarray(w_kv[:, sl]),
            "wv": np.ascontiguousarray(w_kv[:, D + hg * DH:D + (hg + 1) * DH]),
        })
    return in_maps


def make_in_maps_full(x, x_q, w_q, w_kv, w_proj):
    import ml_dtypes

    w_proj = np.asarray(w_proj, dtype=np.float32)
    in_maps = make_in_maps(x, x_q, w_q, w_kv)
    for c in range(NCORES):
        hg = c % 2
        sl = slice(hg * DH, (hg + 1) * DH)
        in_maps[c]["wp"] = np.ascontiguousarray(
            w_proj[sl, :].astype(ml_dtypes.bfloat16)
        )
    return in_maps


def unshard(results, b_proj):
    b_proj = np.asarray(b_proj, dtype=np.float32)
    attn = np.empty((B, H, NQ, NK), dtype=np.float32)
    out = np.empty((B, NQ, D), dtype=np.float32)
    for c in range(NCORES):
        b, hg = c // 2, c % 2
        attn[b, hg * HPC:(hg + 1) * HPC] = results[c]["attn_o"]
    for b in range(B):
        out[b] = results[2 * b]["out_o"] + results[2 * b + 1]["out_o"] + b_proj[None, :]
    return out, attn


def kernel(x, x_q, w_q, w_kv, w_proj, b_proj):
    from concourse.bass_utils import run_bass_kernel_spmd

    nc = get_nc()
    in_maps = make_in_maps_full(x, x_q, w_q, w_kv, w_proj)
    res = run_bass_kernel_spmd(nc, in_maps, list(range(NCORES))).results
    return unshard(res, b_proj)


# revision 15
# speedup vs baseline: 1.1192x; 1.1192x over previous
"""Trainium2 Bass kernel for nn_Attention (B=4, Nq=Nk=1024, D=512, H=8).

Sharding: 8 cores = 4 batches x 2 head-groups (4 heads each).
Core c handles batch b = c // 2, heads [hg*4, hg*4+4) with hg = c % 2.

Per-core device program (all heavy math on device):
  qT = (x_q[b] @ w_q[:, hg])^T          [256, 1024]   (f32r matmuls)
  kT = (x[b]  @ w_k[:, hg])^T           [256, 1024]
  v  =  x[b]  @ w_v[:, hg]              [1024, 256]   (stored bf16)
  per head h (4):
    A: logits[q,k] = qT_h^T kT_h; exp (ACT, scale=0.125 fused, accum row
       sums); normalize in-place (gpsimd, per-partition recip); DMA -> attn
    B: logitsT[k,q] = kT_h^T qT_h; expT = exp (ACT, bf16)
    AV: out_u[q,64] = sum_kj expT_kj^T v_kj (bf16 matmuls);
        scale by recip (DVE, fused into PSUM->SBUF copy)
  transpose out [1024,256] -> outT (PE transposes), proj = outT^T @ w_p
  partial out -> DRAM (host adds the two head-group partials + bias).

Host side: slice/transpose inputs (numpy), run SPMD on 8 cores, concat
attn shards, add the two per-batch partials + b_proj.
"""

import sys

import numpy as np

for _p in ("/opt/trn_rl_repo",):
    if _p not in sys.path:
        sys.path.insert(0, _p)

# Problem constants (hardcoded per contest rules).
B, NQ, NK = 4, 1024, 1024
D = 512          # DIM_Q = DIM_K = OUT_DIM
H = 8
HD = 64          # head dim
SCALE = HD ** -0.5
HPC = 4          # heads per core
DH = HPC * HD    # 256: per-core slice of head dims
P = 128
KO = D // P      # 4 contraction chunks for the input projections
NCORES = 8

_NC_CACHE = {}


def build_nc():
    """Build the (single) SPMD Bass program. Same program on all 8 cores."""
    from contextlib import ExitStack

    import concourse.mybir as mybir
    import concourse.tile as tile
    from concourse import bacc
    from concourse.masks import make_identity

    FP = mybir.dt.float32
    BF = mybir.dt.bfloat16
    FR = mybir.dt.float32r
    Exp = mybir.ActivationFunctionType.Exp

    nc = bacc.Bacc("TRN2")
    xqT = nc.declare_dram_parameter("xqT", [D, NQ], FR, isOutput=False)
    xT = nc.declare_dram_parameter("xT", [D, NK], FR, isOutput=False)
    wq = nc.declare_dram_parameter("wq", [D, DH], FR, isOutput=False)
    wk = nc.declare_dram_parameter("wk", [D, DH], FR, isOutput=False)
    wv = nc.declare_dram_parameter("wv", [D, DH], FR, isOutput=False)
    wp = nc.declare_dram_parameter("wp", [DH, D], BF, isOutput=False)
    attn_o = nc.declare_dram_parameter("attn_o", [HPC, NQ, NK], FP, isOutput=True)
    out_o = nc.declare_dram_parameter("out_o", [NQ, D], FP, isOutput=True)

    with tile.TileContext(nc) as tc:
        with ExitStack() as ctx:
            consts = ctx.enter_context(tc.tile_pool(name="consts", bufs=1))
            persist = ctx.enter_context(tc.tile_pool(name="persist", bufs=1))
            expT_pool = ctx.enter_context(tc.tile_pool(name="expT", bufs=3))
            exA_pool = ctx.enter_context(tc.tile_pool(name="exA", bufs=12))
            outp = ctx.enter_context(tc.tile_pool(name="outp", bufs=2))
            # PSUM budget (8 banks): lg 3x[128,1024]f32 = 6, sm 2x[128,512]f32 = 2
            ps_sm = ctx.enter_context(tc.tile_pool(name="ps_sm", bufs=2, space="PSUM"))
            ps_lg = ctx.enter_context(tc.tile_pool(name="ps_lg", bufs=3, space="PSUM"))

            # ---- load inputs (split big ones per-ko so compute starts early) ----
            xqT_sb = persist.tile([P, KO, NQ], FR)
            xT_sb = persist.tile([P, KO, NK], FR)
            for ko in range(KO):
                nc.sync.dma_start(
                    xqT_sb[:, ko], xqT[:].rearrange("(ko p) q -> ko p q", p=P)[ko])
                nc.sync.dma_start(
                    xT_sb[:, ko], xT[:].rearrange("(ko p) q -> ko p q", p=P)[ko])
            wq_sb = persist.tile([P, KO, DH], FR)
            nc.sync.dma_start(wq_sb[:], wq[:].rearrange("(ko p) m -> p ko m", p=P))
            wk_sb = persist.tile([P, KO, DH], FR)
            nc.sync.dma_start(wk_sb[:], wk[:].rearrange("(ko p) m -> p ko m", p=P))
            wv_sb = persist.tile([P, KO, DH], FR)
            nc.sync.dma_start(wv_sb[:], wv[:].rearrange("(ko p) m -> p ko m", p=P))
            wp_sb = persist.tile([P, 2, D], BF)
            nc.sync.dma_start(wp_sb[:], wp[:].rearrange("(j p) n -> p j n", p=P))
            ident = consts.tile([P, P], BF)
            make_identity(nc, ident[:])

            # ---- projections: qT, kT  ([dh-part(2x128), seq]) ----
            qT_sb = persist.tile([P, 2, NQ], BF)
            kT_sb = persist.tile([P, 2, NK], BF)
            for w_sb, x_sb, dst in ((wq_sb, xqT_sb, qT_sb), (wk_sb, xT_sb, kT_sb)):
                for j in range(2):
                    for qc in range(2):
                        ps = ps_sm.tile([P, 512], FP, tag="sm")
                        for ko in range(KO):
                            nc.tensor.matmul(
                                ps[:],
                                w_sb[:, ko, j * P:(j + 1) * P],
                                x_sb[:, ko, qc * 512:(qc + 1) * 512],
                                start=(ko == 0),
                                stop=(ko == KO - 1),
                            )
                        nc.vector.tensor_copy(dst[:, j, qc * 512:(qc + 1) * 512], ps[:])

            # ---- v in natural layout [k-part, kj, head, hd], bf16 ----
            v_sb = persist.tile([P, 8, HPC, HD], BF)
            for kj in range(8):
                ps = ps_sm.tile([P, DH], FP, tag="sm")
                for ko in range(KO):
                    nc.tensor.matmul(
                        ps[:],
                        xT_sb[:, ko, kj * P:(kj + 1) * P],
                        wv_sb[:, ko, :],
                        start=(ko == 0),
                        stop=(ko == KO - 1),
                    )
                nc.vector.tensor_copy(
                    v_sb[:, kj].rearrange("p h d -> p (h d)"), ps[:]
                )

            # ---- per-head-pair attention ----
            # Heads 2*hp (partitions 0-63) and 2*hp+1 (partitions 64-127) run
            # concurrently in the PE via row-group tiling (tile_position is
            # auto-derived from the lhsT/rhs base partition).
            sums = consts.tile([P, HPC * 8], FP)   # row sums per (head, q-tile)
            rec = consts.tile([P, HPC * 8], FP)    # reciprocals
            out_n = persist.tile([P, 8, DH], BF)   # normalized attn @ v, [q, dh]
            outT = persist.tile([P, 2, NQ], BF)

            for hp in range(2):
                h0, h1 = 2 * hp, 2 * hp + 1
                qkT = []
                for p0 in (0, 64):
                    qkT.append((qT_sb[p0:p0 + 64, hp], kT_sb[p0:p0 + 64, hp]))

                # --- A: logits [q, k] -> exp (+ row sums), normalize, store ---
                exa_tiles = {}
                for mi in range(8):
                    pss = [ps_lg.tile([P, NK], FP, tag="lg", name=f"psA{t}") for t in range(2)]
                    for kc in range(2):
                        for t in range(2):
                            qT_h, kT_h = qkT[t]
                            nc.tensor.matmul(
                                pss[t][:, kc * 512:(kc + 1) * 512],
                                qT_h[:, mi * P:(mi + 1) * P],
                                kT_h[:, kc * 512:(kc + 1) * 512],
                                start=True,
                                stop=True,
                            )
                    for t, h in ((0, h0), (1, h1)):
                        exa = exA_pool.tile([P, NK], FP, tag="exA")
                        si = h * 8 + mi
                        nc.scalar.activation(
                            exa[:], pss[t][:], Exp, scale=SCALE,
                            accum_out=sums[:, si:si + 1],
                        )
                        exa_tiles[(t, mi)] = exa

                    if mi in (3, 7):  # recip + normalize in half-head batches
                        lo, hi = (0, 4) if mi == 3 else (4, 8)
                        for h in (h0, h1):
                            nc.vector.reciprocal(
                                rec[:, h * 8 + lo:h * 8 + hi],
                                sums[:, h * 8 + lo:h * 8 + hi],
                            )
                        for mj in range(lo, hi):
                            for t, h in ((0, h0), (1, h1)):
                                exa = exa_tiles.pop((t, mj))
                                si = h * 8 + mj
                                nc.vector.tensor_scalar_mul(
                                    exa[:], exa[:], rec[:, si:si + 1]
                                )
                                nc.sync.dma_start(
                                    attn_o[h, mj * P:(mj + 1) * P, :], exa[:]
                                )

                # --- B: logitsT [k, q] -> expT (bf16) ---
                expTs = [expT_pool.tile([P, 8, NQ], BF, tag="expT", name=f"expT{t}") for t in range(2)]
                for kj in range(8):
                    pss = [ps_lg.tile([P, NQ], FP, tag="lg", name=f"psB{t}") for t in range(2)]
                    for qc in range(2):
                        for t in range(2):
                            qT_h, kT_h = qkT[t]
                            nc.tensor.matmul(
                                pss[t][:, qc * 512:(qc + 1) * 512],
                                kT_h[:, kj * P:(kj + 1) * P],
                                qT_h[:, qc * 512:(qc + 1) * 512],
                                start=True,
                                stop=True,
                            )
                    for t in range(2):
                        nc.scalar.activation(expTs[t][:, kj], pss[t][:], Exp, scale=SCALE)

                # --- AV: out_u[q, 64] = sum_kj expT_kj^T v_kj; scale by rec ---
                for mi in range(8):
                    for t, h in ((0, h0), (1, h1)):
                        psv = ps_sm.tile([P, HD], FP, tag="sm")
                        for kj in range(8):
                            nc.tensor.matmul(
                                psv[:],
                                expTs[t][:, kj, mi * P:(mi + 1) * P],
                                v_sb[:, kj, h],
                                start=(kj == 0),
                                stop=(kj == 7),
                            )
                        si = h * 8 + mi
                        nc.vector.tensor_scalar_mul(
                            out_n[:, mi, h * HD:(h + 1) * HD], psv[:], rec[:, si:si + 1]
                        )

                # --- transpose this pair's dh block: [q, 128] -> [128, q] ---
                for mi in range(8):
                    pst = ps_sm.tile([P, P], BF, tag="sm")
                    nc.tensor.transpose(
                        pst[:], out_n[:, mi, hp * P:(hp + 1) * P], ident[:]
                    )
                    nc.vector.tensor_copy(outT[:, hp, mi * P:(mi + 1) * P], pst[:])

            # ---- output projection ----
            for mi in range(8):
                ps = ps_sm.tile([P, D], FP, tag="sm")
                for j in range(2):
                    nc.tensor.matmul(
                        ps[:],
                        outT[:, j, mi * P:(mi + 1) * P],
                        wp_sb[:, j, :],
                        start=(j == 0),
                        stop=(j == 1),
                    )
                of = outp.tile([P, D], FP, tag="of")
                nc.vector.tensor_copy(of[:], ps[:])
                nc.sync.dma_start(out_o[mi * P:(mi + 1) * P, :], of[:])

    nc.compile()
    return nc


def get_nc():
    if "nc" not in _NC_CACHE:
        _NC_CACHE["nc"] = build_nc()
    return _NC_CACHE["nc"]


def make_in_maps(x, x_q, w_q, w_kv):
    """Shard full inputs into 8 per-core input maps (host-side numpy)."""
    x = np.asarray(x, dtype=np.float32)
    x_q = np.asarray(x_q, dtype=np.float32)
    w_q = np.asarray(w_q, dtype=np.float32)
    w_kv = np.asarray(w_kv, dtype=np.float32)
    in_maps = []
    for c in range(NCORES):
        b, hg = c // 2, c % 2
        sl = slice(hg * DH, (hg + 1) * DH)
        in_maps.append({
            "xqT": np.ascontiguousarray(x_q[b].T),
            "xT": np.ascontiguousarray(x[b].T),
            "wq": np.ascontiguousarray(w_q[:, sl]),
            "wk": np.ascontiguousarray(w_kv[:, sl]),
            "wv": np.ascontiguousarray(w_kv[:, D + hg * DH:D + (hg + 1) * DH]),
        })
    return in_maps


def make_in_maps_full(x, x_q, w_q, w_kv, w_proj):
    import ml_dtypes

    w_proj = np.asarray(w_proj, dtype=np.float32)
    in_maps = make_in_maps(x, x_q, w_q, w_kv)
    for c in range(NCORES):
        hg = c % 2
        sl = slice(hg * DH, (hg + 1) * DH)
        in_maps[c]["wp"] = np.ascontiguousarray(
            w_proj[sl, :].astype(ml_dtypes.bfloat16)
        )
    return in_maps


def unshard(results, b_proj):
    b_proj = np.asarray(b_proj, dtype=np.float32)
    attn = np.empty((B, H, NQ, NK), dtype=np.float32)
    out = np.empty((B, NQ, D), dtype=np.float32)
    for c in range(NCORES):
        b, hg = c // 2, c % 2
        attn[b, hg * HPC:(hg + 1) * HPC] = results[c]["attn_o"]
    for b in range(B):
        out[b] = results[2 * b]["out_o"] + results[2 * b + 1]["out_o"] + b_proj[None, :]
    return out, attn


def kernel(x, x_q, w_q, w_kv, w_proj, b_proj):
    from concourse.bass_utils import run_bass_kernel_spmd

    nc = get_nc()
    in_maps = make_in_maps_full(x, x_q, w_q, w_kv, w_proj)
    res = run_bass_kernel_spmd(nc, in_maps, list(range(NCORES))).results
    return unshard(res, b_proj)


# revision 23
# speedup vs baseline: 1.1393x; 1.0179x over previous
"""Trainium2 Bass kernel for nn_Attention (B=4, Nq=Nk=1024, D=512, H=8).

Sharding: 8 cores = 4 batches x 2 head-groups (4 heads each).
Core c handles batch b = c // 2, heads [hg*4, hg*4+4) with hg = c % 2.

Per-core device program (all heavy math on device):
  qT = (x_q[b] @ w_q[:, hg])^T          [256, 1024]   (f32r matmuls)
  kT = (x[b]  @ w_k[:, hg])^T           [256, 1024]
  v  =  x[b]  @ w_v[:, hg]              [1024, 256]   (stored bf16)
  per head h (4):
    A: logits[q,k] = qT_h^T kT_h; exp (ACT, scale=0.125 fused, accum row
       sums); normalize in-place (gpsimd, per-partition recip); DMA -> attn
    B: logitsT[k,q] = kT_h^T qT_h; expT = exp (ACT, bf16)
    AV: out_u[q,64] = sum_kj expT_kj^T v_kj (bf16 matmuls);
        scale by recip (DVE, fused into PSUM->SBUF copy)
  transpose out [1024,256] -> outT (PE transposes), proj = outT^T @ w_p
  partial out -> DRAM (host adds the two head-group partials + bias).

Host side: slice/transpose inputs (numpy), run SPMD on 8 cores, concat
attn shards, add the two per-batch partials + b_proj.
"""

import sys

import numpy as np

for _p in ("/opt/trn_rl_repo",):
    if _p not in sys.path:
        sys.path.insert(0, _p)

# Problem constants (hardcoded per contest rules).
B, NQ, NK = 4, 1024, 1024
D = 512          # DIM_Q = DIM_K = OUT_DIM
H = 8
HD = 64          # head dim
SCALE = HD ** -0.5
HPC = 4          # heads per core
DH = HPC * HD    # 256: per-core slice of head dims
P = 128
KO = D // P      # 4 contraction chunks for the input projections
NCORES = 8

_NC_CACHE = {}


def build_nc():
    """Build the (single) SPMD Bass program. Same program on all 8 cores."""
    from contextlib import ExitStack

    import concourse.mybir as mybir
    import concourse.tile as tile
    from concourse import bacc
    from concourse.masks import make_identity

    FP = mybir.dt.float32
    BF = mybir.dt.bfloat16
    FR = mybir.dt.float32r
    Exp = mybir.ActivationFunctionType.Exp

    nc = bacc.Bacc("TRN2")
    xqT = nc.declare_dram_parameter("xqT", [D, NQ], FR, isOutput=False)
    xT = nc.declare_dram_parameter("xT", [D, NK], FR, isOutput=False)
    wq = nc.declare_dram_parameter("wq", [D, DH], FR, isOutput=False)
    wk = nc.declare_dram_parameter("wk", [D, DH], FR, isOutput=False)
    wv = nc.declare_dram_parameter("wv", [D, DH], FR, isOutput=False)
    wp = nc.declare_dram_parameter("wp", [DH, D], BF, isOutput=False)
    attn_o = nc.declare_dram_parameter("attn_o", [HPC, NQ, NK], FP, isOutput=True)
    out_o = [
        nc.declare_dram_parameter(f"out_o{j}", [NQ, D], FP, isOutput=True)
        for j in range(2)
    ]

    with tile.TileContext(nc) as tc:
        with ExitStack() as ctx:
            consts = ctx.enter_context(tc.tile_pool(name="consts", bufs=1))
            persist = ctx.enter_context(tc.tile_pool(name="persist", bufs=1))
            expT_pool = ctx.enter_context(tc.tile_pool(name="expT", bufs=3))
            exA_pool = ctx.enter_context(tc.tile_pool(name="exA", bufs=12))
            outp = ctx.enter_context(tc.tile_pool(name="outp", bufs=2))
            # PSUM budget (8 banks): lg 3x[128,1024]f32 = 6, sm 2x[128,512]f32 = 2
            ps_sm = ctx.enter_context(tc.tile_pool(name="ps_sm", bufs=2, space="PSUM"))
            ps_lg = ctx.enter_context(tc.tile_pool(name="ps_lg", bufs=3, space="PSUM"))

            # ---- load inputs (split so compute starts early, spread queues) ----
            xqT_sb = persist.tile([P, KO, NQ], FR)
            xT_sb = persist.tile([P, KO, NK], FR)
            for ko in range(KO):
                for sh in range(2):
                    nc.sync.dma_start(
                        xqT_sb[:, ko, sh * 512:(sh + 1) * 512],
                        xqT[:].rearrange("(ko p) q -> ko p q", p=P)
                        [ko, :, sh * 512:(sh + 1) * 512])
                    nc.sync.dma_start(
                        xT_sb[:, ko, sh * 512:(sh + 1) * 512],
                        xT[:].rearrange("(ko p) q -> ko p q", p=P)
                        [ko, :, sh * 512:(sh + 1) * 512])
            wq_sb = persist.tile([P, KO, DH], FR)
            nc.sync.dma_start(wq_sb[:], wq[:].rearrange("(ko p) m -> p ko m", p=P))
            wk_sb = persist.tile([P, KO, DH], FR)
            nc.sync.dma_start(wk_sb[:], wk[:].rearrange("(ko p) m -> p ko m", p=P))
            wv_sb = persist.tile([P, KO, DH], FR)
            nc.sync.dma_start(wv_sb[:], wv[:].rearrange("(ko p) m -> p ko m", p=P))
            wp_sb = persist.tile([P, 2, D], BF)
            nc.sync.dma_start(wp_sb[:], wp[:].rearrange("(j p) n -> p j n", p=P))
            ident = consts.tile([P, P], BF)
            make_identity(nc, ident[:])

            # ---- projections: qT, kT  ([dh-part(2x128), seq]) ----
            qT_sb = persist.tile([P, 2, NQ], BF)
            kT_sb = persist.tile([P, 2, NK], BF)
            for w_sb, x_sb, dst in ((wq_sb, xqT_sb, qT_sb), (wk_sb, xT_sb, kT_sb)):
                for j in range(2):
                    for qc in range(2):
                        ps = ps_sm.tile([P, 512], FP, tag="sm")
                        for ko in range(KO):
                            nc.tensor.matmul(
                                ps[:],
                                w_sb[:, ko, j * P:(j + 1) * P],
                                x_sb[:, ko, qc * 512:(qc + 1) * 512],
                                start=(ko == 0),
                                stop=(ko == KO - 1),
                            )
                        nc.vector.tensor_copy(dst[:, j, qc * 512:(qc + 1) * 512], ps[:])

            # ---- v in natural layout [k-part, kj, head, hd], bf16 ----
            v_sb = persist.tile([P, 8, HPC, HD], BF)
            for kj in range(8):
                ps = ps_sm.tile([P, DH], FP, tag="sm")
                for ko in range(KO):
                    nc.tensor.matmul(
                        ps[:],
                        xT_sb[:, ko, kj * P:(kj + 1) * P],
                        wv_sb[:, ko, :],
                        start=(ko == 0),
                        stop=(ko == KO - 1),
                    )
                nc.vector.tensor_copy(
                    v_sb[:, kj].rearrange("p h d -> p (h d)"), ps[:]
                )

            # ---- per-head-pair attention ----
            # Heads 2*hp (partitions 0-63) and 2*hp+1 (partitions 64-127) run
            # concurrently in the PE via row-group tiling (tile_position is
            # auto-derived from the lhsT/rhs base partition).
            sums = consts.tile([P, HPC * 8], FP)   # row sums per (head, q-tile)
            rec = consts.tile([P, HPC * 8], FP)    # reciprocals
            out_n = persist.tile([P, 8, DH], BF)   # normalized attn @ v, [q, dh]
            outT = persist.tile([P, 2, NQ], BF)

            for hp in range(2):
                h0, h1 = 2 * hp, 2 * hp + 1
                expT_of = {}
                for h in (h0, h1):
                    p0 = (h % 2) * 64
                    qT_h = qT_sb[p0:p0 + 64, hp]       # [64, NQ]
                    kT_h = kT_sb[p0:p0 + 64, hp]       # [64, NK]

                    # --- A: logits [q, k] -> exp (+ row sums), norm, store ---
                    exa_tiles = {}
                    for mi in range(8):
                        ps = ps_lg.tile([P, NK], FP, tag="lg")
                        for kc in range(2):
                            nc.tensor.matmul(
                                ps[:, kc * 512:(kc + 1) * 512],
                                qT_h[:, mi * P:(mi + 1) * P],
                                kT_h[:, kc * 512:(kc + 1) * 512],
                                start=True,
                                stop=True,
                            )
                        exa = exA_pool.tile([P, NK], FP, tag="exA")
                        si = h * 8 + mi
                        nc.scalar.activation(
                            exa[:], ps[:], Exp, scale=SCALE,
                            accum_out=sums[:, si:si + 1],
                        )
                        exa_tiles[mi] = exa

                        if mi in (3, 7):  # recip + normalize in half batches
                            lo, hi = (0, 4) if mi == 3 else (4, 8)
                            nc.vector.reciprocal(
                                rec[:, h * 8 + lo:h * 8 + hi],
                                sums[:, h * 8 + lo:h * 8 + hi],
                            )
                            for mj in range(lo, hi):
                                exa = exa_tiles.pop(mj)
                                sj = h * 8 + mj
                                nc.vector.tensor_scalar_mul(
                                    exa[:], exa[:], rec[:, sj:sj + 1]
                                )
                                nc.sync.dma_start(
                                    attn_o[h, mj * P:(mj + 1) * P, :], exa[:]
                                )

                    # --- B: logitsT [k, q] -> expT (bf16) ---
                    expT = expT_pool.tile([P, 8, NQ], BF, tag="expT")
                    expT_of[h] = expT
                    for kj in range(8):
                        ps = ps_lg.tile([P, NQ], FP, tag="lg")
                        for qc in range(2):
                            nc.tensor.matmul(
                                ps[:, qc * 512:(qc + 1) * 512],
                                kT_h[:, kj * P:(kj + 1) * P],
                                qT_h[:, qc * 512:(qc + 1) * 512],
                                start=True,
                                stop=True,
                            )
                        nc.scalar.activation(expT[:, kj], ps[:], Exp, scale=SCALE)

                # --- AV: out_u[q, 64] = sum_kj expT_kj^T v_kj; scale by rec ---
                for mi in range(8):
                    for h in (h0, h1):
                        psv = ps_sm.tile([P, HD], FP, tag="sm")
                        for kj in range(8):
                            nc.tensor.matmul(
                                psv[:],
                                expT_of[h][:, kj, mi * P:(mi + 1) * P],
                                v_sb[:, kj, h],
                                start=(kj == 0),
                                stop=(kj == 7),
                            )
                        si = h * 8 + mi
                        nc.vector.tensor_scalar_mul(
                            out_n[:, mi, h * HD:(h + 1) * HD], psv[:], rec[:, si:si + 1]
                        )

                # --- transpose this pair's dh block: [q, 128] -> [128, q] ---
                for mi in range(8):
                    pst = ps_sm.tile([P, P], BF, tag="sm")
                    nc.tensor.transpose(
                        pst[:], out_n[:, mi, hp * P:(hp + 1) * P], ident[:]
                    )
                    nc.vector.tensor_copy(outT[:, hp, mi * P:(mi + 1) * P], pst[:])

                # --- this pair's half of the output projection ---
                # Host adds the two partials (along with the other core's),
                # so the pair-0 projection runs in the shadow of pair 1.
                for mi in range(8):
                    ps = ps_sm.tile([P, D], FP, tag="sm")
                    nc.tensor.matmul(
                        ps[:],
                        outT[:, hp, mi * P:(mi + 1) * P],
                        wp_sb[:, hp, :],
                        start=True,
                        stop=True,
                    )
                    of = outp.tile([P, D], FP, tag="of")
                    nc.vector.tensor_copy(of[:], ps[:])
                    nc.sync.dma_start(out_o[hp][mi * P:(mi + 1) * P, :], of[:])

    nc.compile()
    return nc


def get_nc():
    if "nc" not in _NC_CACHE:
        _NC_CACHE["nc"] = build_nc()
    return _NC_CACHE["nc"]


def make_in_maps(x, x_q, w_q, w_kv):
    """Shard full inputs into 8 per-core input maps (host-side numpy)."""
    x = np.asarray(x, dtype=np.float32)
    x_q = np.asarray(x_q, dtype=np.float32)
    w_q = np.asarray(w_q, dtype=np.float32)
    w_kv = np.asarray(w_kv, dtype=np.float32)
    in_maps = []
    for c in range(NCORES):
        b, hg = c // 2, c % 2
        sl = slice(hg * DH, (hg + 1) * DH)
        in_maps.append({
            "xqT": np.ascontiguousarray(x_q[b].T),
            "xT": np.ascontiguousarray(x[b].T),
            "wq": np.ascontiguousarray(w_q[:, sl]),
            "wk": np.ascontiguousarray(w_kv[:, sl]),
            "wv": np.ascontiguousarray(w_kv[:, D + hg * DH:D + (hg + 1) * DH]),
        })
    return in_maps


def make_in_maps_full(x, x_q, w_q, w_kv, w_proj):
    import ml_dtypes

    w_proj = np.asarray(w_proj, dtype=np.float32)
    in_maps = make_in_maps(x, x_q, w_q, w_kv)
    for c in range(NCORES):
        hg = c % 2
        sl = slice(hg * DH, (hg + 1) * DH)
        in_maps[c]["wp"] = np.ascontiguousarray(
            w_proj[sl, :].astype(ml_dtypes.bfloat16)
        )
    return in_maps


def unshard(results, b_proj):
    b_proj = np.asarray(b_proj, dtype=np.float32)
    attn = np.empty((B, H, NQ, NK), dtype=np.float32)
    out = np.empty((B, NQ, D), dtype=np.float32)
    for c in range(NCORES):
        b, hg = c // 2, c % 2
        attn[b, hg * HPC:(hg + 1) * HPC] = results[c]["attn_o"]
    for b in range(B):
        out[b] = (
            results[2 * b]["out_o0"] + results[2 * b]["out_o1"]
            + results[2 * b + 1]["out_o0"] + results[2 * b + 1]["out_o1"]
            + b_proj[None, :]
        )
    return out, attn


def kernel(x, x_q, w_q, w_kv, w_proj, b_proj):
    from concourse.bass_utils import run_bass_kernel_spmd

    nc = get_nc()
    in_maps = make_in_maps_full(x, x_q, w_q, w_kv, w_proj)
    res = run_bass_kernel_spmd(nc, in_maps, list(range(NCORES))).results
    return unshard(res, b_proj)


# revision 26
# speedup vs baseline: 1.1569x; 1.0155x over previous
"""Trainium2 Bass kernel for nn_Attention (B=4, Nq=Nk=1024, D=512, H=8).

Sharding: 8 cores = 4 batches x 2 head-groups (4 heads each).
Core c handles batch b = c // 2, heads [hg*4, hg*4+4) with hg = c % 2.

Per-core device program (all heavy math on device):
  qT = (x_q[b] @ w_q[:, hg])^T          [256, 1024]   (f32r matmuls)
  kT = (x[b]  @ w_k[:, hg])^T           [256, 1024]
  v  =  x[b]  @ w_v[:, hg]              [1024, 256]   (stored bf16)
  per head h (4):
    A: logits[q,k] = qT_h^T kT_h; exp (ACT, scale=0.125 fused, accum row
       sums); normalize in-place (gpsimd, per-partition recip); DMA -> attn
    B: logitsT[k,q] = kT_h^T qT_h; expT = exp (ACT, bf16)
    AV: out_u[q,64] = sum_kj expT_kj^T v_kj (bf16 matmuls);
        scale by recip (DVE, fused into PSUM->SBUF copy)
  transpose out [1024,256] -> outT (PE transposes), proj = outT^T @ w_p
  partial out -> DRAM (host adds the two head-group partials + bias).

Host side: slice/transpose inputs (numpy), run SPMD on 8 cores, concat
attn shards, add the two per-batch partials + b_proj.
"""

import sys

import numpy as np

for _p in ("/opt/trn_rl_repo",):
    if _p not in sys.path:
        sys.path.insert(0, _p)

# Problem constants (hardcoded per contest rules).
B, NQ, NK = 4, 1024, 1024
D = 512          # DIM_Q = DIM_K = OUT_DIM
H = 8
HD = 64          # head dim
SCALE = HD ** -0.5
HPC = 4          # heads per core
DH = HPC * HD    # 256: per-core slice of head dims
P = 128
KO = D // P      # 4 contraction chunks for the input projections
NCORES = 8

_NC_CACHE = {}


def build_nc():
    """Build the (single) SPMD Bass program. Same program on all 8 cores."""
    from contextlib import ExitStack

    import concourse.mybir as mybir
    import concourse.tile as tile
    from concourse import bacc
    from concourse.masks import make_identity

    FP = mybir.dt.float32
    BF = mybir.dt.bfloat16
    FR = mybir.dt.float32r
    Exp = mybir.ActivationFunctionType.Exp

    nc = bacc.Bacc("TRN2")
    xqT = nc.declare_dram_parameter("xqT", [D, NQ], BF, isOutput=False)
    xT = nc.declare_dram_parameter("xT", [D, NK], BF, isOutput=False)
    wq = nc.declare_dram_parameter("wq", [D, DH], BF, isOutput=False)
    wk = nc.declare_dram_parameter("wk", [D, DH], BF, isOutput=False)
    wv = nc.declare_dram_parameter("wv", [D, DH], BF, isOutput=False)
    wp = nc.declare_dram_parameter("wp", [DH, D], BF, isOutput=False)
    attn_o = nc.declare_dram_parameter("attn_o", [HPC, NQ, NK], FP, isOutput=True)
    out_o = [
        nc.declare_dram_parameter(f"out_o{j}", [NQ, D], FP, isOutput=True)
        for j in range(2)
    ]

    with tile.TileContext(nc) as tc:
        with ExitStack() as ctx:
            consts = ctx.enter_context(tc.tile_pool(name="consts", bufs=1))
            persist = ctx.enter_context(tc.tile_pool(name="persist", bufs=1))
            expT_pool = ctx.enter_context(tc.tile_pool(name="expT", bufs=3))
            exA_pool = ctx.enter_context(tc.tile_pool(name="exA", bufs=12))
            outp = ctx.enter_context(tc.tile_pool(name="outp", bufs=2))
            # PSUM budget (8 banks): lg 3x[128,1024]f32 = 6, sm 2x[128,512]f32 = 2
            ps_sm = ctx.enter_context(tc.tile_pool(name="ps_sm", bufs=2, space="PSUM"))
            ps_lg = ctx.enter_context(tc.tile_pool(name="ps_lg", bufs=3, space="PSUM"))

            # ---- load inputs (split so compute starts early, spread queues) ----
            xqT_sb = persist.tile([P, KO, NQ], BF)
            xT_sb = persist.tile([P, KO, NK], BF)
            for ko in range(KO):
                for sh in range(2):
                    nc.sync.dma_start(
                        xqT_sb[:, ko, sh * 512:(sh + 1) * 512],
                        xqT[:].rearrange("(ko p) q -> ko p q", p=P)
                        [ko, :, sh * 512:(sh + 1) * 512])
                    nc.sync.dma_start(
                        xT_sb[:, ko, sh * 512:(sh + 1) * 512],
                        xT[:].rearrange("(ko p) q -> ko p q", p=P)
                        [ko, :, sh * 512:(sh + 1) * 512])
            wq_sb = persist.tile([P, KO, DH], BF)
            nc.sync.dma_start(wq_sb[:], wq[:].rearrange("(ko p) m -> p ko m", p=P))
            wk_sb = persist.tile([P, KO, DH], BF)
            nc.sync.dma_start(wk_sb[:], wk[:].rearrange("(ko p) m -> p ko m", p=P))
            wv_sb = persist.tile([P, KO, DH], BF)
            nc.sync.dma_start(wv_sb[:], wv[:].rearrange("(ko p) m -> p ko m", p=P))
            wp_sb = persist.tile([P, 2, D], BF)
            nc.sync.dma_start(wp_sb[:], wp[:].rearrange("(j p) n -> p j n", p=P))
            ident = consts.tile([P, P], BF)
            make_identity(nc, ident[:])

            # ---- projections: qT, kT  ([dh-part(2x128), seq]) ----
            qT_sb = persist.tile([P, 2, NQ], BF)
            kT_sb = persist.tile([P, 2, NK], BF)
            for j in range(2):
                for w_sb, x_sb, dst in ((wq_sb, xqT_sb, qT_sb), (wk_sb, xT_sb, kT_sb)):
                    for qc in range(2):
                        ps = ps_sm.tile([P, 512], FP, tag="sm")
                        for ko in range(KO):
                            nc.tensor.matmul(
                                ps[:],
                                w_sb[:, ko, j * P:(j + 1) * P],
                                x_sb[:, ko, qc * 512:(qc + 1) * 512],
                                start=(ko == 0),
                                stop=(ko == KO - 1),
                            )
                        nc.vector.tensor_copy(dst[:, j, qc * 512:(qc + 1) * 512], ps[:])

            # ---- v in natural layout [k-part, kj, head, hd], bf16 ----
            v_sb = persist.tile([P, 8, HPC, HD], BF)
            for kj in range(8):
                ps = ps_sm.tile([P, DH], FP, tag="sm")
                for ko in range(KO):
                    nc.tensor.matmul(
                        ps[:],
                        xT_sb[:, ko, kj * P:(kj + 1) * P],
                        wv_sb[:, ko, :],
                        start=(ko == 0),
                        stop=(ko == KO - 1),
                    )
                nc.vector.tensor_copy(
                    v_sb[:, kj].rearrange("p h d -> p (h d)"), ps[:]
                )

            # ---- per-head-pair attention ----
            # Heads 2*hp (partitions 0-63) and 2*hp+1 (partitions 64-127) run
            # concurrently in the PE via row-group tiling (tile_position is
            # auto-derived from the lhsT/rhs base partition).
            sums = consts.tile([P, HPC * 8], FP)   # row sums per (head, q-tile)
            rec = consts.tile([P, HPC * 8], FP)    # reciprocals
            out_n = persist.tile([P, 8, DH], BF)   # normalized attn @ v, [q, dh]
            outT = persist.tile([P, 2, NQ], BF)

            for hp in range(2):
                h0, h1 = 2 * hp, 2 * hp + 1
                for h in (h0, h1):
                    p0 = (h % 2) * 64
                    qT_h = qT_sb[p0:p0 + 64, hp]       # [64, NQ]
                    kT_h = kT_sb[p0:p0 + 64, hp]       # [64, NK]

                    # --- A: logits [q, k] -> exp (+ row sums), norm, store ---
                    exa_tiles = {}
                    for mi in range(8):
                        ps = ps_lg.tile([P, NK], FP, tag="lg")
                        for kc in range(2):
                            nc.tensor.matmul(
                                ps[:, kc * 512:(kc + 1) * 512],
                                qT_h[:, mi * P:(mi + 1) * P],
                                kT_h[:, kc * 512:(kc + 1) * 512],
                                start=True,
                                stop=True,
                            )
                        exa = exA_pool.tile([P, NK], FP, tag="exA")
                        si = h * 8 + mi
                        nc.scalar.activation(
                            exa[:], ps[:], Exp, scale=SCALE,
                            accum_out=sums[:, si:si + 1],
                        )
                        exa_tiles[mi] = exa

                        if mi in (3, 7):  # recip + normalize in half batches
                            lo, hi = (0, 4) if mi == 3 else (4, 8)
                            nc.vector.reciprocal(
                                rec[:, h * 8 + lo:h * 8 + hi],
                                sums[:, h * 8 + lo:h * 8 + hi],
                            )
                            for mj in range(lo, hi):
                                exa = exa_tiles.pop(mj)
                                sj = h * 8 + mj
                                nc.vector.tensor_scalar_mul(
                                    exa[:], exa[:], rec[:, sj:sj + 1]
                                )
                                nc.sync.dma_start(
                                    attn_o[h, mj * P:(mj + 1) * P, :], exa[:]
                                )

                    # --- B: logitsT [k, q] -> expT (bf16) ---
                    expT = expT_pool.tile([P, 8, NQ], BF, tag="expT")
                    for kj in range(8):
                        ps = ps_lg.tile([P, NQ], FP, tag="lg")
                        for qc in range(2):
                            nc.tensor.matmul(
                                ps[:, qc * 512:(qc + 1) * 512],
                                kT_h[:, kj * P:(kj + 1) * P],
                                qT_h[:, qc * 512:(qc + 1) * 512],
                                start=True,
                                stop=True,
                            )
                        nc.scalar.activation(expT[:, kj], ps[:], Exp, scale=SCALE)

                    # --- AV: out_u[q,64] = sum_kj expT_kj^T v_kj; scale ---
                    for mi in range(8):
                        psv = ps_sm.tile([P, HD], FP, tag="sm")
                        for kj in range(8):
                            nc.tensor.matmul(
                                psv[:],
                                expT[:, kj, mi * P:(mi + 1) * P],
                                v_sb[:, kj, h],
                                start=(kj == 0),
                                stop=(kj == 7),
                            )
                        si = h * 8 + mi
                        nc.vector.tensor_scalar_mul(
                            out_n[:, mi, h * HD:(h + 1) * HD], psv[:],
                            rec[:, si:si + 1],
                        )

                # --- transpose this pair's dh block: [q, 128] -> [128, q] ---
                for mi in range(8):
                    pst = ps_sm.tile([P, P], BF, tag="sm")
                    nc.tensor.transpose(
                        pst[:], out_n[:, mi, hp * P:(hp + 1) * P], ident[:]
                    )
                    nc.vector.tensor_copy(outT[:, hp, mi * P:(mi + 1) * P], pst[:])

                # --- this pair's half of the output projection ---
                # Host adds the two partials (along with the other core's),
                # so the pair-0 projection runs in the shadow of pair 1.
                for mi in range(8):
                    ps = ps_sm.tile([P, D], FP, tag="sm")
                    nc.tensor.matmul(
                        ps[:],
                        outT[:, hp, mi * P:(mi + 1) * P],
                        wp_sb[:, hp, :],
                        start=True,
                        stop=True,
                    )
                    of = outp.tile([P, D], FP, tag="of")
                    nc.vector.tensor_copy(of[:], ps[:])
                    nc.sync.dma_start(out_o[hp][mi * P:(mi + 1) * P, :], of[:])

    nc.compile()
    return nc


def get_nc():
    if "nc" not in _NC_CACHE:
        _NC_CACHE["nc"] = build_nc()
    return _NC_CACHE["nc"]


def make_in_maps(x, x_q, w_q, w_kv):
    """Shard full inputs into 8 per-core input maps (host-side numpy)."""
    import ml_dtypes

    bf = ml_dtypes.bfloat16
    x = np.asarray(x, dtype=np.float32)
    x_q = np.asarray(x_q, dtype=np.float32)
    w_q = np.asarray(w_q, dtype=np.float32)
    w_kv = np.asarray(w_kv, dtype=np.float32)
    xqT_b = [np.ascontiguousarray(x_q[b].T.astype(bf)) for b in range(B)]
    xT_b = [np.ascontiguousarray(x[b].T.astype(bf)) for b in range(B)]
    in_maps = []
    for c in range(NCORES):
        b, hg = c // 2, c % 2
        sl = slice(hg * DH, (hg + 1) * DH)
        in_maps.append({
            "xqT": xqT_b[b],
            "xT": xT_b[b],
            "wq": np.ascontiguousarray(w_q[:, sl].astype(bf)),
            "wk": np.ascontiguousarray(w_kv[:, sl].astype(bf)),
            "wv": np.ascontiguousarray(
                w_kv[:, D + hg * DH:D + (hg + 1) * DH].astype(bf)),
        })
    return in_maps


def make_in_maps_full(x, x_q, w_q, w_kv, w_proj):
    import ml_dtypes

    w_proj = np.asarray(w_proj, dtype=np.float32)
    in_maps = make_in_maps(x, x_q, w_q, w_kv)
    for c in range(NCORES):
        hg = c % 2
        sl = slice(hg * DH, (hg + 1) * DH)
        in_maps[c]["wp"] = np.ascontiguousarray(
            w_proj[sl, :].astype(ml_dtypes.bfloat16)
        )
    return in_maps


def unshard(results, b_proj):
    b_proj = np.asarray(b_proj, dtype=np.float32)
    attn = np.empty((B, H, NQ, NK), dtype=np.float32)
    out = np.empty((B, NQ, D), dtype=np.float32)
    for c in range(NCORES):
        b, hg = c // 2, c % 2
        attn[b, hg * HPC:(hg + 1) * HPC] = results[c]["attn_o"]
    for b in range(B):
        out[b] = (
            results[2 * b]["out_o0"] + results[2 * b]["out_o1"]
            + results[2 * b + 1]["out_o0"] + results[2 * b + 1]["out_o1"]
            + b_proj[None, :]
        )
    return out, attn


def kernel(x, x_q, w_q, w_kv, w_proj, b_proj):
    from concourse.bass_utils import run_bass_kernel_spmd

    nc = get_nc()
    in_maps = make_in_maps_full(x, x_q, w_q, w_kv, w_proj)
    res = run_bass_kernel_spmd(nc, in_maps, list(range(NCORES))).results
    return unshard(res, b_proj)


# revision 27
# speedup vs baseline: 1.1950x; 1.0329x over previous
"""Trainium2 Bass kernel for nn_Attention (B=4, Nq=Nk=1024, D=512, H=8).

Sharding: 8 cores = 4 batches x 2 head-groups (4 heads each).
Core c handles batch b = c // 2, heads [hg*4, hg*4+4) with hg = c % 2.

Per-core device program (all heavy math on device):
  qT = (x_q[b] @ w_q[:, hg])^T          [256, 1024]   (f32r matmuls)
  kT = (x[b]  @ w_k[:, hg])^T           [256, 1024]
  v  =  x[b]  @ w_v[:, hg]              [1024, 256]   (stored bf16)
  per head h (4):
    A: logits[q,k] = qT_h^T kT_h; exp (ACT, scale=0.125 fused, accum row
       sums); normalize in-place (gpsimd, per-partition recip); DMA -> attn
    B: logitsT[k,q] = kT_h^T qT_h; expT = exp (ACT, bf16)
    AV: out_u[q,64] = sum_kj expT_kj^T v_kj (bf16 matmuls);
        scale by recip (DVE, fused into PSUM->SBUF copy)
  transpose out [1024,256] -> outT (PE transposes), proj = outT^T @ w_p
  partial out -> DRAM (host adds the two head-group partials + bias).

Host side: slice/transpose inputs (numpy), run SPMD on 8 cores, concat
attn shards, add the two per-batch partials + b_proj.
"""

import sys

import numpy as np

for _p in ("/opt/trn_rl_repo",):
    if _p not in sys.path:
        sys.path.insert(0, _p)

# Problem constants (hardcoded per contest rules).
B, NQ, NK = 4, 1024, 1024
D = 512          # DIM_Q = DIM_K = OUT_DIM
H = 8
HD = 64          # head dim
SCALE = HD ** -0.5
HPC = 4          # heads per core
DH = HPC * HD    # 256: per-core slice of head dims
P = 128
KO = D // P      # 4 contraction chunks for the input projections
NCORES = 8

_NC_CACHE = {}


def build_nc():
    """Build the (single) SPMD Bass program. Same program on all 8 cores."""
    from contextlib import ExitStack

    import concourse.mybir as mybir
    import concourse.tile as tile
    from concourse import bacc
    from concourse.masks import make_identity

    FP = mybir.dt.float32
    BF = mybir.dt.bfloat16
    FR = mybir.dt.float32r
    Exp = mybir.ActivationFunctionType.Exp

    nc = bacc.Bacc("TRN2")
    xqT = nc.declare_dram_parameter("xqT", [D, NQ], BF, isOutput=False)
    xT = nc.declare_dram_parameter("xT", [D, NK], BF, isOutput=False)
    wq = nc.declare_dram_parameter("wq", [D, DH], BF, isOutput=False)
    wk = nc.declare_dram_parameter("wk", [D, DH], BF, isOutput=False)
    wv = nc.declare_dram_parameter("wv", [D, DH], BF, isOutput=False)
    wp = nc.declare_dram_parameter("wp", [DH, D], BF, isOutput=False)
    attn_o = nc.declare_dram_parameter("attn_o", [HPC, NQ, NK], FP, isOutput=True)
    out_o = [
        nc.declare_dram_parameter(f"out_o{j}", [NQ, D], FP, isOutput=True)
        for j in range(2)
    ]

    with tile.TileContext(nc) as tc:
        with ExitStack() as ctx:
            consts = ctx.enter_context(tc.tile_pool(name="consts", bufs=1))
            persist = ctx.enter_context(tc.tile_pool(name="persist", bufs=1))
            expT_pool = ctx.enter_context(tc.tile_pool(name="expT", bufs=3))
            exA_pool = ctx.enter_context(tc.tile_pool(name="exA", bufs=12))
            outp = ctx.enter_context(tc.tile_pool(name="outp", bufs=2))
            # PSUM budget (8 banks): lg 3x[128,1024]f32 = 6, sm 2x[128,512]f32 = 2
            ps_sm = ctx.enter_context(tc.tile_pool(name="ps_sm", bufs=2, space="PSUM"))
            ps_lg = ctx.enter_context(tc.tile_pool(name="ps_lg", bufs=3, space="PSUM"))

            # ---- load inputs (split so compute starts early, spread queues) ----
            xqT_sb = persist.tile([P, KO, NQ], BF)
            xT_sb = persist.tile([P, KO, NK], BF)
            for ko in range(KO):
                for sh in range(2):
                    nc.sync.dma_start(
                        xqT_sb[:, ko, sh * 512:(sh + 1) * 512],
                        xqT[:].rearrange("(ko p) q -> ko p q", p=P)
                        [ko, :, sh * 512:(sh + 1) * 512])
                    nc.sync.dma_start(
                        xT_sb[:, ko, sh * 512:(sh + 1) * 512],
                        xT[:].rearrange("(ko p) q -> ko p q", p=P)
                        [ko, :, sh * 512:(sh + 1) * 512])
            wq_sb = persist.tile([P, KO, DH], BF)
            nc.sync.dma_start(wq_sb[:], wq[:].rearrange("(ko p) m -> p ko m", p=P))
            wk_sb = persist.tile([P, KO, DH], BF)
            nc.sync.dma_start(wk_sb[:], wk[:].rearrange("(ko p) m -> p ko m", p=P))
            wv_sb = persist.tile([P, KO, DH], BF)
            nc.sync.dma_start(wv_sb[:], wv[:].rearrange("(ko p) m -> p ko m", p=P))
            wp_sb = persist.tile([P, 2, D], BF)
            nc.sync.dma_start(wp_sb[:], wp[:].rearrange("(j p) n -> p j n", p=P))
            ident = consts.tile([P, P], BF)
            make_identity(nc, ident[:])

            # ---- projections: qT, kT  ([dh-part(2x128), seq]) ----
            qT_sb = persist.tile([P, 2, NQ], BF)
            kT_sb = persist.tile([P, 2, NK], BF)
            for j in range(2):
                for w_sb, x_sb, dst in ((wq_sb, xqT_sb, qT_sb), (wk_sb, xT_sb, kT_sb)):
                    for qc in range(2):
                        ps = ps_sm.tile([P, 512], FP, tag="sm")
                        for ko in range(KO):
                            nc.tensor.matmul(
                                ps[:],
                                w_sb[:, ko, j * P:(j + 1) * P],
                                x_sb[:, ko, qc * 512:(qc + 1) * 512],
                                start=(ko == 0),
                                stop=(ko == KO - 1),
                            )
                        nc.vector.tensor_copy(dst[:, j, qc * 512:(qc + 1) * 512], ps[:])

            # ---- v in natural layout [k-part, kj, head, hd], bf16 ----
            v_sb = persist.tile([P, 8, HPC, HD], BF)
            for kj in range(8):
                ps = ps_sm.tile([P, DH], FP, tag="sm")
                for ko in range(KO):
                    nc.tensor.matmul(
                        ps[:],
                        xT_sb[:, ko, kj * P:(kj + 1) * P],
                        wv_sb[:, ko, :],
                        start=(ko == 0),
                        stop=(ko == KO - 1),
                    )
                nc.vector.tensor_copy(
                    v_sb[:, kj].rearrange("p h d -> p (h d)"), ps[:]
                )

            # ---- per-head-pair attention ----
            # Heads 2*hp (partitions 0-63) and 2*hp+1 (partitions 64-127) run
            # concurrently in the PE via row-group tiling (tile_position is
            # auto-derived from the lhsT/rhs base partition).
            sums = consts.tile([P, HPC * 8], FP)   # row sums per (head, q-tile)
            rec = consts.tile([P, HPC * 8], FP)    # reciprocals
            out_n = persist.tile([P, 8, DH], BF)   # normalized attn @ v, [q, dh]
            outT = persist.tile([P, 2, NQ], BF)

            for hp in range(2):
                h0, h1 = 2 * hp, 2 * hp + 1
                for h in (h0, h1):
                    p0 = (h % 2) * 64
                    qT_h = qT_sb[p0:p0 + 64, hp]       # [64, NQ]
                    kT_h = kT_sb[p0:p0 + 64, hp]       # [64, NK]

                    # --- A: logits [q, k] -> exp (+ row sums), norm, store ---
                    exa_tiles = {}
                    for mi in range(8):
                        ps = ps_lg.tile([P, NK], FP, tag="lg")
                        for kc in range(2):
                            nc.tensor.matmul(
                                ps[:, kc * 512:(kc + 1) * 512],
                                qT_h[:, mi * P:(mi + 1) * P],
                                kT_h[:, kc * 512:(kc + 1) * 512],
                                start=True,
                                stop=True,
                            )
                        exa = exA_pool.tile([P, NK], FP, tag="exA")
                        si = h * 8 + mi
                        nc.scalar.activation(
                            exa[:], ps[:], Exp, scale=SCALE,
                            accum_out=sums[:, si:si + 1],
                        )
                        exa_tiles[mi] = exa

                        if mi in (3, 7):  # recip + normalize in half batches
                            lo, hi = (0, 4) if mi == 3 else (4, 8)
                            nc.vector.reciprocal(
                                rec[:, h * 8 + lo:h * 8 + hi],
                                sums[:, h * 8 + lo:h * 8 + hi],
                            )
                            for mj in range(lo, hi):
                                exa = exa_tiles.pop(mj)
                                sj = h * 8 + mj
                                nc.vector.tensor_scalar_mul(
                                    exa[:], exa[:], rec[:, sj:sj + 1]
                                )
                                nc.sync.dma_start(
                                    attn_o[h, mj * P:(mj + 1) * P, :], exa[:]
                                )

                    # --- B: logitsT [k, q] -> expT (bf16) ---
                    expT = expT_pool.tile([P, 8, NQ], BF, tag="expT")
                    for kj in range(8):
                        ps = ps_lg.tile([P, NQ], FP, tag="lg")
                        for qc in range(2):
                            nc.tensor.matmul(
                                ps[:, qc * 512:(qc + 1) * 512],
                                kT_h[:, kj * P:(kj + 1) * P],
                                qT_h[:, qc * 512:(qc + 1) * 512],
                                start=True,
                                stop=True,
                            )
                        nc.scalar.activation(expT[:, kj], ps[:], Exp, scale=SCALE)

                    # --- AV: out_u[q,64] = sum_kj expT_kj^T v_kj; scale ---
                    # All 8 q-tiles of this head accumulate into one PSUM
                    # bank; one broadcast-multiply normalizes and evacuates.
                    psv = ps_sm.tile([P, 8, HD], FP, tag="sm")
                    for mi in range(8):
                        for kj in range(8):
                            nc.tensor.matmul(
                                psv[:, mi],
                                expT[:, kj, mi * P:(mi + 1) * P],
                                v_sb[:, kj, h],
                                start=(kj == 0),
                                stop=(kj == 7),
                            )
                    nc.vector.tensor_tensor(
                        out_n[:, :, h * HD:(h + 1) * HD],
                        psv[:],
                        rec[:, h * 8:(h + 1) * 8].unsqueeze(-1).to_broadcast(
                            [P, 8, HD]),
                        mybir.AluOpType.mult,
                    )

                # --- transpose this pair's dh block: [q, 128] -> [128, q] ---
                for mi in range(8):
                    pst = ps_sm.tile([P, P], BF, tag="sm")
                    nc.tensor.transpose(
                        pst[:], out_n[:, mi, hp * P:(hp + 1) * P], ident[:]
                    )
                    nc.vector.tensor_copy(outT[:, hp, mi * P:(mi + 1) * P], pst[:])

                # --- this pair's half of the output projection ---
                # Host adds the two partials (along with the other core's),
                # so the pair-0 projection runs in the shadow of pair 1.
                for mi in range(8):
                    ps = ps_sm.tile([P, D], FP, tag="sm")
                    nc.tensor.matmul(
                        ps[:],
                        outT[:, hp, mi * P:(mi + 1) * P],
                        wp_sb[:, hp, :],
                        start=True,
                        stop=True,
                    )
                    of = outp.tile([P, D], FP, tag="of")
                    nc.vector.tensor_copy(of[:], ps[:])
                    nc.sync.dma_start(out_o[hp][mi * P:(mi + 1) * P, :], of[:])

    nc.compile()
    return nc


def get_nc():
    if "nc" not in _NC_CACHE:
        _NC_CACHE["nc"] = build_nc()
    return _NC_CACHE["nc"]


def make_in_maps(x, x_q, w_q, w_kv):
    """Shard full inputs into 8 per-core input maps (host-side numpy)."""
    import ml_dtypes

    bf = ml_dtypes.bfloat16
    x = np.asarray(x, dtype=np.float32)
    x_q = np.asarray(x_q, dtype=np.float32)
    w_q = np.asarray(w_q, dtype=np.float32)
    w_kv = np.asarray(w_kv, dtype=np.float32)
    xqT_b = [np.ascontiguousarray(x_q[b].T.astype(bf)) for b in range(B)]
    xT_b = [np.ascontiguousarray(x[b].T.astype(bf)) for b in range(B)]
    in_maps = []
    for c in range(NCORES):
        b, hg = c // 2, c % 2
        sl = slice(hg * DH, (hg + 1) * DH)
        in_maps.append({
            "xqT": xqT_b[b],
            "xT": xT_b[b],
            "wq": np.ascontiguousarray(w_q[:, sl].astype(bf)),
            "wk": np.ascontiguousarray(w_kv[:, sl].astype(bf)),
            "wv": np.ascontiguousarray(
                w_kv[:, D + hg * DH:D + (hg + 1) * DH].astype(bf)),
        })
    return in_maps


def make_in_maps_full(x, x_q, w_q, w_kv, w_proj):
    import ml_dtypes

    w_proj = np.asarray(w_proj, dtype=np.float32)
    in_maps = make_in_maps(x, x_q, w_q, w_kv)
    for c in range(NCORES):
        hg = c % 2
        sl = slice(hg * DH, (hg + 1) * DH)
        in_maps[c]["wp"] = np.ascontiguousarray(
            w_proj[sl, :].astype(ml_dtypes.bfloat16)
        )
    return in_maps


def unshard(results, b_proj):
    b_proj = np.asarray(b_proj, dtype=np.float32)
    attn = np.empty((B, H, NQ, NK), dtype=np.float32)
    out = np.empty((B, NQ, D), dtype=np.float32)
    for c in range(NCORES):
        b, hg = c // 2, c % 2
        attn[b, hg * HPC:(hg + 1) * HPC] = results[c]["attn_o"]
    for b in range(B):
        out[b] = (
            results[2 * b]["out_o0"] + results[2 * b]["out_o1"]
            + results[2 * b + 1]["out_o0"] + results[2 * b + 1]["out_o1"]
            + b_proj[None, :]
        )
    return out, attn


def kernel(x, x_q, w_q, w_kv, w_proj, b_proj):
    from concourse.bass_utils import run_bass_kernel_spmd

    nc = get_nc()
    in_maps = make_in_maps_full(x, x_q, w_q, w_kv, w_proj)
    res = run_bass_kernel_spmd(nc, in_maps, list(range(NCORES))).results
    return unshard(res, b_proj)


# revision 28
# speedup vs baseline: 1.2102x; 1.0127x over previous
"""Trainium2 Bass kernel for nn_Attention (B=4, Nq=Nk=1024, D=512, H=8).

Sharding: 8 cores = 4 batches x 2 head-groups (4 heads each).
Core c handles batch b = c // 2, heads [hg*4, hg*4+4) with hg = c % 2.

Per-core device program (all heavy math on device):
  qT = (x_q[b] @ w_q[:, hg])^T          [256, 1024]   (f32r matmuls)
  kT = (x[b]  @ w_k[:, hg])^T           [256, 1024]
  v  =  x[b]  @ w_v[:, hg]              [1024, 256]   (stored bf16)
  per head h (4):
    A: logits[q,k] = qT_h^T kT_h; exp (ACT, scale=0.125 fused, accum row
       sums); normalize in-place (gpsimd, per-partition recip); DMA -> attn
    B: logitsT[k,q] = kT_h^T qT_h; expT = exp (ACT, bf16)
    AV: out_u[q,64] = sum_kj expT_kj^T v_kj (bf16 matmuls);
        scale by recip (DVE, fused into PSUM->SBUF copy)
  transpose out [1024,256] -> outT (PE transposes), proj = outT^T @ w_p
  partial out -> DRAM (host adds the two head-group partials + bias).

Host side: slice/transpose inputs (numpy), run SPMD on 8 cores, concat
attn shards, add the two per-batch partials + b_proj.
"""

import sys

import numpy as np

for _p in ("/opt/trn_rl_repo",):
    if _p not in sys.path:
        sys.path.insert(0, _p)

# Problem constants (hardcoded per contest rules).
B, NQ, NK = 4, 1024, 1024
D = 512          # DIM_Q = DIM_K = OUT_DIM
H = 8
HD = 64          # head dim
SCALE = HD ** -0.5
HPC = 4          # heads per core
DH = HPC * HD    # 256: per-core slice of head dims
P = 128
KO = D // P      # 4 contraction chunks for the input projections
NCORES = 8

_NC_CACHE = {}


def build_nc():
    """Build the (single) SPMD Bass program. Same program on all 8 cores."""
    from contextlib import ExitStack

    import concourse.mybir as mybir
    import concourse.tile as tile
    from concourse import bacc
    from concourse.masks import make_identity

    FP = mybir.dt.float32
    BF = mybir.dt.bfloat16
    FR = mybir.dt.float32r
    Exp = mybir.ActivationFunctionType.Exp

    nc = bacc.Bacc("TRN2")
    xqT = nc.declare_dram_parameter("xqT", [D, NQ], BF, isOutput=False)
    xT = nc.declare_dram_parameter("xT", [D, NK], BF, isOutput=False)
    wq = nc.declare_dram_parameter("wq", [D, DH], BF, isOutput=False)
    wk = nc.declare_dram_parameter("wk", [D, DH], BF, isOutput=False)
    wv = nc.declare_dram_parameter("wv", [D, DH], BF, isOutput=False)
    wp = nc.declare_dram_parameter("wp", [DH, D], BF, isOutput=False)
    attn_o = nc.declare_dram_parameter("attn_o", [HPC, NQ, NK], FP, isOutput=True)
    out_o = [
        nc.declare_dram_parameter(f"out_o{j}", [NQ, D], FP, isOutput=True)
        for j in range(2)
    ]

    with tile.TileContext(nc) as tc:
        with ExitStack() as ctx:
            consts = ctx.enter_context(tc.tile_pool(name="consts", bufs=1))
            persist = ctx.enter_context(tc.tile_pool(name="persist", bufs=1))
            expT_pool = ctx.enter_context(tc.tile_pool(name="expT", bufs=3))
            exA_pool = ctx.enter_context(tc.tile_pool(name="exA", bufs=12))
            outp = ctx.enter_context(tc.tile_pool(name="outp", bufs=2))
            # PSUM budget (8 banks): lg 3x[128,1024]f32 = 6, sm 2x[128,512]f32 = 2
            ps_sm = ctx.enter_context(tc.tile_pool(name="ps_sm", bufs=2, space="PSUM"))
            ps_lg = ctx.enter_context(tc.tile_pool(name="ps_lg", bufs=3, space="PSUM"))

            # ---- load inputs (split so compute starts early, spread queues) ----
            xqT_sb = persist.tile([P, KO, NQ], BF)
            xT_sb = persist.tile([P, KO, NK], BF)
            for ko in range(KO):
                for sh in range(2):
                    nc.sync.dma_start(
                        xqT_sb[:, ko, sh * 512:(sh + 1) * 512],
                        xqT[:].rearrange("(ko p) q -> ko p q", p=P)
                        [ko, :, sh * 512:(sh + 1) * 512])
                    nc.sync.dma_start(
                        xT_sb[:, ko, sh * 512:(sh + 1) * 512],
                        xT[:].rearrange("(ko p) q -> ko p q", p=P)
                        [ko, :, sh * 512:(sh + 1) * 512])
            wq_sb = persist.tile([P, KO, DH], BF)
            nc.sync.dma_start(wq_sb[:], wq[:].rearrange("(ko p) m -> p ko m", p=P))
            wk_sb = persist.tile([P, KO, DH], BF)
            nc.sync.dma_start(wk_sb[:], wk[:].rearrange("(ko p) m -> p ko m", p=P))
            wv_sb = persist.tile([P, KO, DH], BF)
            nc.sync.dma_start(wv_sb[:], wv[:].rearrange("(ko p) m -> p ko m", p=P))
            wp_sb = persist.tile([P, 2, D], BF)
            nc.sync.dma_start(wp_sb[:], wp[:].rearrange("(j p) n -> p j n", p=P))
            ident = consts.tile([P, P], BF)
            make_identity(nc, ident[:])

            # ---- projections: qT, kT  ([dh-part(2x128), seq]) ----
            qT_sb = persist.tile([P, 2, NQ], BF)
            kT_sb = persist.tile([P, 2, NK], BF)
            for j in range(2):
                for w_sb, x_sb, dst in ((wq_sb, xqT_sb, qT_sb), (wk_sb, xT_sb, kT_sb)):
                    for qc in range(2):
                        ps = ps_sm.tile([P, 512], FP, tag="sm")
                        for ko in range(KO):
                            nc.tensor.matmul(
                                ps[:],
                                w_sb[:, ko, j * P:(j + 1) * P],
                                x_sb[:, ko, qc * 512:(qc + 1) * 512],
                                start=(ko == 0),
                                stop=(ko == KO - 1),
                            )
                        nc.vector.tensor_copy(dst[:, j, qc * 512:(qc + 1) * 512], ps[:])

            # ---- v in natural layout [k-part, kj, head, hd], bf16 ----
            v_sb = persist.tile([P, 8, HPC, HD], BF)
            for kj in range(8):
                ps = ps_sm.tile([P, DH], FP, tag="sm")
                for ko in range(KO):
                    nc.tensor.matmul(
                        ps[:],
                        xT_sb[:, ko, kj * P:(kj + 1) * P],
                        wv_sb[:, ko, :],
                        start=(ko == 0),
                        stop=(ko == KO - 1),
                    )
                nc.vector.tensor_copy(
                    v_sb[:, kj].rearrange("p h d -> p (h d)"), ps[:]
                )

            # ---- per-head-pair attention ----
            # Heads 2*hp (partitions 0-63) and 2*hp+1 (partitions 64-127) run
            # concurrently in the PE via row-group tiling (tile_position is
            # auto-derived from the lhsT/rhs base partition).
            sums = consts.tile([P, HPC * 8], FP)   # row sums per (head, q-tile)
            rec = consts.tile([P, HPC * 8], FP)    # reciprocals
            out_n = persist.tile([P, 8, DH], BF)   # normalized attn @ v, [q, dh]
            outT = persist.tile([P, 2, NQ], BF)

            for hp in range(2):
                h0, h1 = 2 * hp, 2 * hp + 1
                for h in (h0, h1):
                    p0 = (h % 2) * 64
                    qT_h = qT_sb[p0:p0 + 64, hp]       # [64, NQ]
                    kT_h = kT_sb[p0:p0 + 64, hp]       # [64, NK]

                    # --- A+B interleaved: two independent PE->ACT streams ---
                    # A: logits [q, k] -> exp f32 (+ row sums) -> norm -> DMA
                    # B: logitsT [k, q] -> expT bf16 (feeds AV)
                    exa_tiles = {}
                    expT = expT_pool.tile([P, 8, NQ], BF, tag="expT")
                    for i in range(8):
                        psa = ps_lg.tile([P, NK], FP, tag="lg", name="psa")
                        for kc in range(2):
                            nc.tensor.matmul(
                                psa[:, kc * 512:(kc + 1) * 512],
                                qT_h[:, i * P:(i + 1) * P],
                                kT_h[:, kc * 512:(kc + 1) * 512],
                                start=True,
                                stop=True,
                            )
                        exa = exA_pool.tile([P, NK], FP, tag="exA")
                        si = h * 8 + i
                        nc.scalar.activation(
                            exa[:], psa[:], Exp, scale=SCALE,
                            accum_out=sums[:, si:si + 1],
                        )
                        exa_tiles[i] = exa

                        psb = ps_lg.tile([P, NQ], FP, tag="lg", name="psb")
                        for qc in range(2):
                            nc.tensor.matmul(
                                psb[:, qc * 512:(qc + 1) * 512],
                                kT_h[:, i * P:(i + 1) * P],
                                qT_h[:, qc * 512:(qc + 1) * 512],
                                start=True,
                                stop=True,
                            )
                        nc.scalar.activation(expT[:, i], psb[:], Exp, scale=SCALE)

                        if i in (3, 7):  # recip + normalize in half batches
                            lo, hi = (0, 4) if i == 3 else (4, 8)
                            nc.vector.reciprocal(
                                rec[:, h * 8 + lo:h * 8 + hi],
                                sums[:, h * 8 + lo:h * 8 + hi],
                            )
                            for mj in range(lo, hi):
                                exa = exa_tiles.pop(mj)
                                sj = h * 8 + mj
                                nc.vector.tensor_scalar_mul(
                                    exa[:], exa[:], rec[:, sj:sj + 1]
                                )
                                nc.sync.dma_start(
                                    attn_o[h, mj * P:(mj + 1) * P, :], exa[:]
                                )

                    # --- AV: out_u[q,64] = sum_kj expT_kj^T v_kj; scale ---
                    # All 8 q-tiles of this head accumulate into one PSUM
                    # bank; one broadcast-multiply normalizes and evacuates.
                    psv = ps_sm.tile([P, 8, HD], FP, tag="sm")
                    for mi in range(8):
                        for kj in range(8):
                            nc.tensor.matmul(
                                psv[:, mi],
                                expT[:, kj, mi * P:(mi + 1) * P],
                                v_sb[:, kj, h],
                                start=(kj == 0),
                                stop=(kj == 7),
                            )
                    nc.vector.tensor_tensor(
                        out_n[:, :, h * HD:(h + 1) * HD],
                        psv[:],
                        rec[:, h * 8:(h + 1) * 8].unsqueeze(-1).to_broadcast(
                            [P, 8, HD]),
                        mybir.AluOpType.mult,
                    )

                # --- transpose this pair's dh block: [q, 128] -> [128, q] ---
                for mi in range(8):
                    pst = ps_sm.tile([P, P], BF, tag="sm")
                    nc.tensor.transpose(
                        pst[:], out_n[:, mi, hp * P:(hp + 1) * P], ident[:]
                    )
                    nc.vector.tensor_copy(outT[:, hp, mi * P:(mi + 1) * P], pst[:])

                # --- this pair's half of the output projection ---
                # Host adds the two partials (along with the other core's),
                # so the pair-0 projection runs in the shadow of pair 1.
                for mi in range(8):
                    ps = ps_sm.tile([P, D], FP, tag="sm")
                    nc.tensor.matmul(
                        ps[:],
                        outT[:, hp, mi * P:(mi + 1) * P],
                        wp_sb[:, hp, :],
                        start=True,
                        stop=True,
                    )
                    of = outp.tile([P, D], FP, tag="of")
                    nc.vector.tensor_copy(of[:], ps[:])
                    nc.sync.dma_start(out_o[hp][mi * P:(mi + 1) * P, :], of[:])

    nc.compile()
    return nc


def get_nc():
    if "nc" not in _NC_CACHE:
        _NC_CACHE["nc"] = build_nc()
    return _NC_CACHE["nc"]


def make_in_maps(x, x_q, w_q, w_kv):
    """Shard full inputs into 8 per-core input maps (host-side numpy)."""
    import ml_dtypes

    bf = ml_dtypes.bfloat16
    x = np.asarray(x, dtype=np.float32)
    x_q = np.asarray(x_q, dtype=np.float32)
    w_q = np.asarray(w_q, dtype=np.float32)
    w_kv = np.asarray(w_kv, dtype=np.float32)
    xqT_b = [np.ascontiguousarray(x_q[b].T.astype(bf)) for b in range(B)]
    xT_b = [np.ascontiguousarray(x[b].T.astype(bf)) for b in range(B)]
    in_maps = []
    for c in range(NCORES):
        b, hg = c // 2, c % 2
        sl = slice(hg * DH, (hg + 1) * DH)
        in_maps.append({
            "xqT": xqT_b[b],
            "xT": xT_b[b],
            "wq": np.ascontiguousarray(w_q[:, sl].astype(bf)),
            "wk": np.ascontiguousarray(w_kv[:, sl].astype(bf)),
            "wv": np.ascontiguousarray(
                w_kv[:, D + hg * DH:D + (hg + 1) * DH].astype(bf)),
        })
    return in_maps


def make_in_maps_full(x, x_q, w_q, w_kv, w_proj):
    import ml_dtypes

    w_proj = np.asarray(w_proj, dtype=np.float32)
    in_maps = make_in_maps(x, x_q, w_q, w_kv)
    for c in range(NCORES):
        hg = c % 2
        sl = slice(hg * DH, (hg + 1) * DH)
        in_maps[c]["wp"] = np.ascontiguousarray(
            w_proj[sl, :].astype(ml_dtypes.bfloat16)
        )
    return in_maps


def unshard(results, b_proj):
    b_proj = np.asarray(b_proj, dtype=np.float32)
    attn = np.empty((B, H, NQ, NK), dtype=np.float32)
    out = np.empty((B, NQ, D), dtype=np.float32)
    for c in range(NCORES):
        b, hg = c // 2, c % 2
        attn[b, hg * HPC:(hg + 1) * HPC] = results[c]["attn_o"]
    for b in range(B):
        out[b] = (
            results[2 * b]["out_o0"] + results[2 * b]["out_o1"]
            + results[2 * b + 1]["out_o0"] + results[2 * b + 1]["out_o1"]
            + b_proj[None, :]
        )
    return out, attn


def kernel(x, x_q, w_q, w_kv, w_proj, b_proj):
    from concourse.bass_utils import run_bass_kernel_spmd

    nc = get_nc()
    in_maps = make_in_maps_full(x, x_q, w_q, w_kv, w_proj)
    res = run_bass_kernel_spmd(nc, in_maps, list(range(NCORES))).results
    return unshard(res, b_proj)


# revision 29
# speedup vs baseline: 1.2352x; 1.0207x over previous
"""Trainium2 Bass kernel for nn_Attention (B=4, Nq=Nk=1024, D=512, H=8).

Sharding: 8 cores = 4 batches x 2 head-groups (4 heads each).
Core c handles batch b = c // 2, heads [hg*4, hg*4+4) with hg = c % 2.

Per-core device program (all heavy math on device):
  qT = (x_q[b] @ w_q[:, hg])^T          [256, 1024]   (f32r matmuls)
  kT = (x[b]  @ w_k[:, hg])^T           [256, 1024]
  v  =  x[b]  @ w_v[:, hg]              [1024, 256]   (stored bf16)
  per head h (4):
    A: logits[q,k] = qT_h^T kT_h; exp (ACT, scale=0.125 fused, accum row
       sums); normalize in-place (gpsimd, per-partition recip); DMA -> attn
    B: logitsT[k,q] = kT_h^T qT_h; expT = exp (ACT, bf16)
    AV: out_u[q,64] = sum_kj expT_kj^T v_kj (bf16 matmuls);
        scale by recip (DVE, fused into PSUM->SBUF copy)
  transpose out [1024,256] -> outT (PE transposes), proj = outT^T @ w_p
  partial out -> DRAM (host adds the two head-group partials + bias).

Host side: slice/transpose inputs (numpy), run SPMD on 8 cores, concat
attn shards, add the two per-batch partials + b_proj.
"""

import sys

import numpy as np

for _p in ("/opt/trn_rl_repo",):
    if _p not in sys.path:
        sys.path.insert(0, _p)

# Problem constants (hardcoded per contest rules).
B, NQ, NK = 4, 1024, 1024
D = 512          # DIM_Q = DIM_K = OUT_DIM
H = 8
HD = 64          # head dim
SCALE = HD ** -0.5
HPC = 4          # heads per core
DH = HPC * HD    # 256: per-core slice of head dims
P = 128
KO = D // P      # 4 contraction chunks for the input projections
NCORES = 8

_NC_CACHE = {}


def build_nc():
    """Build the (single) SPMD Bass program. Same program on all 8 cores."""
    from contextlib import ExitStack

    import concourse.mybir as mybir
    import concourse.tile as tile
    from concourse import bacc
    from concourse.masks import make_identity

    FP = mybir.dt.float32
    BF = mybir.dt.bfloat16
    FR = mybir.dt.float32r
    Exp = mybir.ActivationFunctionType.Exp

    nc = bacc.Bacc("TRN2")
    xqT = nc.declare_dram_parameter("xqT", [D, NQ], BF, isOutput=False)
    xT = nc.declare_dram_parameter("xT", [D, NK], BF, isOutput=False)
    wq = nc.declare_dram_parameter("wq", [D, DH], BF, isOutput=False)
    wk = nc.declare_dram_parameter("wk", [D, DH], BF, isOutput=False)
    wv = nc.declare_dram_parameter("wv", [D, DH], BF, isOutput=False)
    wp = nc.declare_dram_parameter("wp", [DH, D], BF, isOutput=False)
    attn_o = nc.declare_dram_parameter("attn_o", [HPC, NQ, NK], FP, isOutput=True)
    out_o = [
        nc.declare_dram_parameter(f"out_o{j}", [NQ, D], FP, isOutput=True)
        for j in range(2)
    ]

    with tile.TileContext(nc) as tc:
        with ExitStack() as ctx:
            consts = ctx.enter_context(tc.tile_pool(name="consts", bufs=1))
            persist = ctx.enter_context(tc.tile_pool(name="persist", bufs=1))
            expT_pool = ctx.enter_context(tc.tile_pool(name="expT", bufs=3))
            exA_pool = ctx.enter_context(tc.tile_pool(name="exA", bufs=12))
            outp = ctx.enter_context(tc.tile_pool(name="outp", bufs=2))
            # PSUM budget (8 banks): lg 3x[128,1024]f32 = 6, sm 2x[128,512]f32 = 2
            ps_sm = ctx.enter_context(tc.tile_pool(name="ps_sm", bufs=2, space="PSUM"))
            ps_lg = ctx.enter_context(tc.tile_pool(name="ps_lg", bufs=3, space="PSUM"))

            # ---- load inputs (split so compute starts early, spread queues) ----
            xqT_sb = persist.tile([P, KO, NQ], BF)
            xT_sb = persist.tile([P, KO, NK], BF)
            for ko in range(KO):
                for sh in range(2):
                    nc.sync.dma_start(
                        xqT_sb[:, ko, sh * 512:(sh + 1) * 512],
                        xqT[:].rearrange("(ko p) q -> ko p q", p=P)
                        [ko, :, sh * 512:(sh + 1) * 512])
                    nc.sync.dma_start(
                        xT_sb[:, ko, sh * 512:(sh + 1) * 512],
                        xT[:].rearrange("(ko p) q -> ko p q", p=P)
                        [ko, :, sh * 512:(sh + 1) * 512])
            wq_sb = persist.tile([P, KO, DH], BF)
            nc.sync.dma_start(wq_sb[:], wq[:].rearrange("(ko p) m -> p ko m", p=P))
            wk_sb = persist.tile([P, KO, DH], BF)
            nc.sync.dma_start(wk_sb[:], wk[:].rearrange("(ko p) m -> p ko m", p=P))
            wv_sb = persist.tile([P, KO, DH], BF)
            nc.sync.dma_start(wv_sb[:], wv[:].rearrange("(ko p) m -> p ko m", p=P))
            wp_sb = persist.tile([P, 2, D], BF)
            nc.sync.dma_start(wp_sb[:], wp[:].rearrange("(j p) n -> p j n", p=P))
            ident = consts.tile([P, P], BF)
            make_identity(nc, ident[:])

            # ---- PE warm-up burst during the input-DMA window ----
            # The HAM clock gate keeps the PE at 1.2 GHz until it sees ~3.4us
            # of sustained matmul activity; without this burst every matmul in
            # the kernel runs at half clock. Junk matmuls on the identity tile
            # keep the PE busy from ~7us (identity ready) until the input DMAs
            # land, so the real matmuls start at 2.4 GHz and stay there.
            psw = ps_lg.tile([P, P], FP, tag="lg", name="warm")
            for _ in range(100):
                nc.tensor.matmul(psw[:], ident[:], ident[:], start=True, stop=True)

            # ---- projections: qT, kT  ([dh-part(2x128), seq]) ----
            qT_sb = persist.tile([P, 2, NQ], BF)
            kT_sb = persist.tile([P, 2, NK], BF)
            for j in range(2):
                for w_sb, x_sb, dst in ((wq_sb, xqT_sb, qT_sb), (wk_sb, xT_sb, kT_sb)):
                    for qc in range(2):
                        ps = ps_sm.tile([P, 512], FP, tag="sm")
                        for ko in range(KO):
                            nc.tensor.matmul(
                                ps[:],
                                w_sb[:, ko, j * P:(j + 1) * P],
                                x_sb[:, ko, qc * 512:(qc + 1) * 512],
                                start=(ko == 0),
                                stop=(ko == KO - 1),
                            )
                        nc.vector.tensor_copy(dst[:, j, qc * 512:(qc + 1) * 512], ps[:])

            # ---- v in natural layout [k-part, kj, head, hd], bf16 ----
            v_sb = persist.tile([P, 8, HPC, HD], BF)
            for kj in range(8):
                ps = ps_sm.tile([P, DH], FP, tag="sm")
                for ko in range(KO):
                    nc.tensor.matmul(
                        ps[:],
                        xT_sb[:, ko, kj * P:(kj + 1) * P],
                        wv_sb[:, ko, :],
                        start=(ko == 0),
                        stop=(ko == KO - 1),
                    )
                nc.vector.tensor_copy(
                    v_sb[:, kj].rearrange("p h d -> p (h d)"), ps[:]
                )

            # ---- per-head-pair attention ----
            # Heads 2*hp (partitions 0-63) and 2*hp+1 (partitions 64-127) run
            # concurrently in the PE via row-group tiling (tile_position is
            # auto-derived from the lhsT/rhs base partition).
            sums = consts.tile([P, HPC * 8], FP)   # row sums per (head, q-tile)
            rec = consts.tile([P, HPC * 8], FP)    # reciprocals
            out_n = persist.tile([P, 8, DH], BF)   # normalized attn @ v, [q, dh]
            outT = persist.tile([P, 2, NQ], BF)

            for hp in range(2):
                h0, h1 = 2 * hp, 2 * hp + 1
                for h in (h0, h1):
                    p0 = (h % 2) * 64
                    qT_h = qT_sb[p0:p0 + 64, hp]       # [64, NQ]
                    kT_h = kT_sb[p0:p0 + 64, hp]       # [64, NK]

                    # --- A+B interleaved: two independent PE->ACT streams ---
                    # A: logits [q, k] -> exp f32 (+ row sums) -> norm -> DMA
                    # B: logitsT [k, q] -> expT bf16 (feeds AV)
                    exa_tiles = {}
                    expT = expT_pool.tile([P, 8, NQ], BF, tag="expT")
                    for i in range(8):
                        psa = ps_lg.tile([P, NK], FP, tag="lg", name="psa")
                        for kc in range(2):
                            nc.tensor.matmul(
                                psa[:, kc * 512:(kc + 1) * 512],
                                qT_h[:, i * P:(i + 1) * P],
                                kT_h[:, kc * 512:(kc + 1) * 512],
                                start=True,
                                stop=True,
                            )
                        exa = exA_pool.tile([P, NK], FP, tag="exA")
                        si = h * 8 + i
                        nc.scalar.activation(
                            exa[:], psa[:], Exp, scale=SCALE,
                            accum_out=sums[:, si:si + 1],
                        )
                        exa_tiles[i] = exa

                        psb = ps_lg.tile([P, NQ], FP, tag="lg", name="psb")
                        for qc in range(2):
                            nc.tensor.matmul(
                                psb[:, qc * 512:(qc + 1) * 512],
                                kT_h[:, i * P:(i + 1) * P],
                                qT_h[:, qc * 512:(qc + 1) * 512],
                                start=True,
                                stop=True,
                            )
                        nc.scalar.activation(expT[:, i], psb[:], Exp, scale=SCALE)

                        if i in (3, 7):  # recip + normalize in half batches
                            lo, hi = (0, 4) if i == 3 else (4, 8)
                            nc.vector.reciprocal(
                                rec[:, h * 8 + lo:h * 8 + hi],
                                sums[:, h * 8 + lo:h * 8 + hi],
                            )
                            for mj in range(lo, hi):
                                exa = exa_tiles.pop(mj)
                                sj = h * 8 + mj
                                nc.vector.tensor_scalar_mul(
                                    exa[:], exa[:], rec[:, sj:sj + 1]
                                )
                                nc.sync.dma_start(
                                    attn_o[h, mj * P:(mj + 1) * P, :], exa[:]
                                )

                    # --- AV: out_u[q,64] = sum_kj expT_kj^T v_kj; scale ---
                    # All 8 q-tiles of this head accumulate into one PSUM
                    # bank; one broadcast-multiply normalizes and evacuates.
                    psv = ps_sm.tile([P, 8, HD], FP, tag="sm")
                    for mi in range(8):
                        for kj in range(8):
                            nc.tensor.matmul(
                                psv[:, mi],
                                expT[:, kj, mi * P:(mi + 1) * P],
                                v_sb[:, kj, h],
                                start=(kj == 0),
                                stop=(kj == 7),
                            )
                    nc.vector.tensor_tensor(
                        out_n[:, :, h * HD:(h + 1) * HD],
                        psv[:],
                        rec[:, h * 8:(h + 1) * 8].unsqueeze(-1).to_broadcast(
                            [P, 8, HD]),
                        mybir.AluOpType.mult,
                    )

                # --- transpose this pair's dh block: [q, 128] -> [128, q] ---
                for mi in range(8):
                    pst = ps_sm.tile([P, P], BF, tag="sm")
                    nc.tensor.transpose(
                        pst[:], out_n[:, mi, hp * P:(hp + 1) * P], ident[:]
                    )
                    nc.vector.tensor_copy(outT[:, hp, mi * P:(mi + 1) * P], pst[:])

                # --- this pair's half of the output projection ---
                # Host adds the two partials (along with the other core's),
                # so the pair-0 projection runs in the shadow of pair 1.
                for mi in range(8):
                    ps = ps_sm.tile([P, D], FP, tag="sm")
                    nc.tensor.matmul(
                        ps[:],
                        outT[:, hp, mi * P:(mi + 1) * P],
                        wp_sb[:, hp, :],
                        start=True,
                        stop=True,
                    )
                    of = outp.tile([P, D], FP, tag="of")
                    nc.vector.tensor_copy(of[:], ps[:])
                    nc.sync.dma_start(out_o[hp][mi * P:(mi + 1) * P, :], of[:])

    nc.compile()
    return nc


def get_nc():
    if "nc" not in _NC_CACHE:
        _NC_CACHE["nc"] = build_nc()
    return _NC_CACHE["nc"]


def make_in_maps(x, x_q, w_q, w_kv):
    """Shard full inputs into 8 per-core input maps (host-side numpy)."""
    import ml_dtypes

    bf = ml_dtypes.bfloat16
    x = np.asarray(x, dtype=np.float32)
    x_q = np.asarray(x_q, dtype=np.float32)
    w_q = np.asarray(w_q, dtype=np.float32)
    w_kv = np.asarray(w_kv, dtype=np.float32)
    xqT_b = [np.ascontiguousarray(x_q[b].T.astype(bf)) for b in range(B)]
    xT_b = [np.ascontiguousarray(x[b].T.astype(bf)) for b in range(B)]
    in_maps = []
    for c in range(NCORES):
        b, hg = c // 2, c % 2
        sl = slice(hg * DH, (hg + 1) * DH)
        in_maps.append({
            "xqT": xqT_b[b],
            "xT": xT_b[b],
            "wq": np.ascontiguousarray(w_q[:, sl].astype(bf)),
            "wk": np.ascontiguousarray(w_kv[:, sl].astype(bf)),
            "wv": np.ascontiguousarray(
                w_kv[:, D + hg * DH:D + (hg + 1) * DH].astype(bf)),
        })
    return in_maps


def make_in_maps_full(x, x_q, w_q, w_kv, w_proj):
    import ml_dtypes

    w_proj = np.asarray(w_proj, dtype=np.float32)
    in_maps = make_in_maps(x, x_q, w_q, w_kv)
    for c in range(NCORES):
        hg = c % 2
        sl = slice(hg * DH, (hg + 1) * DH)
        in_maps[c]["wp"] = np.ascontiguousarray(
            w_proj[sl, :].astype(ml_dtypes.bfloat16)
        )
    return in_maps


def unshard(results, b_proj):
    b_proj = np.asarray(b_proj, dtype=np.float32)
    attn = np.empty((B, H, NQ, NK), dtype=np.float32)
    out = np.empty((B, NQ, D), dtype=np.float32)
    for c in range(NCORES):
        b, hg = c // 2, c % 2
        attn[b, hg * HPC:(hg + 1) * HPC] = results[c]["attn_o"]
    for b in range(B):
        out[b] = (
            results[2 * b]["out_o0"] + results[2 * b]["out_o1"]
            + results[2 * b + 1]["out_o0"] + results[2 * b + 1]["out_o1"]
            + b_proj[None, :]
        )
    return out, attn


def kernel(x, x_q, w_q, w_kv, w_proj, b_proj):
    from concourse.bass_utils import run_bass_kernel_spmd

    nc = get_nc()
    in_maps = make_in_maps_full(x, x_q, w_q, w_kv, w_proj)
    res = run_bass_kernel_spmd(nc, in_maps, list(range(NCORES))).results
    return unshard(res, b_proj)


# revision 31
# speedup vs baseline: 1.2448x; 1.0078x over previous
"""Trainium2 Bass kernel for nn_Attention (B=4, Nq=Nk=1024, D=512, H=8).

Sharding: 8 cores = 4 batches x 2 head-groups (4 heads each).
Core c handles batch b = c // 2, heads [hg*4, hg*4+4) with hg = c % 2.

Per-core device program (all heavy math on device):
  qT = (x_q[b] @ w_q[:, hg])^T          [256, 1024]   (f32r matmuls)
  kT = (x[b]  @ w_k[:, hg])^T           [256, 1024]
  v  =  x[b]  @ w_v[:, hg]              [1024, 256]   (stored bf16)
  per head h (4):
    A: logits[q,k] = qT_h^T kT_h; exp (ACT, scale=0.125 fused, accum row
       sums); normalize in-place (gpsimd, per-partition recip); DMA -> attn
    B: logitsT[k,q] = kT_h^T qT_h; expT = exp (ACT, bf16)
    AV: out_u[q,64] = sum_kj expT_kj^T v_kj (bf16 matmuls);
        scale by recip (DVE, fused into PSUM->SBUF copy)
  transpose out [1024,256] -> outT (PE transposes), proj = outT^T @ w_p
  partial out -> DRAM (host adds the two head-group partials + bias).

Host side: slice/transpose inputs (numpy), run SPMD on 8 cores, concat
attn shards, add the two per-batch partials + b_proj.
"""

import sys

import numpy as np

for _p in ("/opt/trn_rl_repo",):
    if _p not in sys.path:
        sys.path.insert(0, _p)

# Problem constants (hardcoded per contest rules).
B, NQ, NK = 4, 1024, 1024
D = 512          # DIM_Q = DIM_K = OUT_DIM
H = 8
HD = 64          # head dim
SCALE = HD ** -0.5
HPC = 4          # heads per core
DH = HPC * HD    # 256: per-core slice of head dims
P = 128
KO = D // P      # 4 contraction chunks for the input projections
NCORES = 8

_NC_CACHE = {}


def build_nc():
    """Build the (single) SPMD Bass program. Same program on all 8 cores."""
    from contextlib import ExitStack

    import concourse.mybir as mybir
    import concourse.tile as tile
    from concourse import bacc
    from concourse.masks import make_identity

    FP = mybir.dt.float32
    BF = mybir.dt.bfloat16
    FR = mybir.dt.float32r
    Exp = mybir.ActivationFunctionType.Exp

    nc = bacc.Bacc("TRN2")
    xqT = nc.declare_dram_parameter("xqT", [D, NQ], BF, isOutput=False)
    xT = nc.declare_dram_parameter("xT", [D, NK], BF, isOutput=False)
    wq = nc.declare_dram_parameter("wq", [D, DH], BF, isOutput=False)
    wk = nc.declare_dram_parameter("wk", [D, DH], BF, isOutput=False)
    wv = nc.declare_dram_parameter("wv", [D, DH], BF, isOutput=False)
    wp = nc.declare_dram_parameter("wp", [DH, D], BF, isOutput=False)
    attn_o = nc.declare_dram_parameter("attn_o", [HPC, NQ, NK], FP, isOutput=True)
    out_o = [
        nc.declare_dram_parameter(f"out_o{j}", [NQ, D], FP, isOutput=True)
        for j in range(2)
    ]

    with tile.TileContext(nc) as tc:
        with ExitStack() as ctx:
            consts = ctx.enter_context(tc.tile_pool(name="consts", bufs=1))
            persist = ctx.enter_context(tc.tile_pool(name="persist", bufs=1))
            expT_pool = ctx.enter_context(tc.tile_pool(name="expT", bufs=3))
            exA_pool = ctx.enter_context(tc.tile_pool(name="exA", bufs=12))
            outp = ctx.enter_context(tc.tile_pool(name="outp", bufs=2))
            # PSUM budget (8 banks): lg 3x[128,1024]f32 = 6, sm 2x[128,512]f32 = 2
            ps_sm = ctx.enter_context(tc.tile_pool(name="ps_sm", bufs=2, space="PSUM"))
            ps_lg = ctx.enter_context(tc.tile_pool(name="ps_lg", bufs=3, space="PSUM"))

            # ---- load inputs (split so compute starts early, spread queues) ----
            xqT_sb = persist.tile([P, KO, NQ], BF)
            xT_sb = persist.tile([P, KO, NK], BF)
            for ko in range(KO):
                for sh in range(2):
                    nc.sync.dma_start(
                        xqT_sb[:, ko, sh * 512:(sh + 1) * 512],
                        xqT[:].rearrange("(ko p) q -> ko p q", p=P)
                        [ko, :, sh * 512:(sh + 1) * 512])
                    nc.sync.dma_start(
                        xT_sb[:, ko, sh * 512:(sh + 1) * 512],
                        xT[:].rearrange("(ko p) q -> ko p q", p=P)
                        [ko, :, sh * 512:(sh + 1) * 512])
            wq_sb = persist.tile([P, KO, DH], BF)
            nc.sync.dma_start(wq_sb[:], wq[:].rearrange("(ko p) m -> p ko m", p=P))
            wk_sb = persist.tile([P, KO, DH], BF)
            nc.sync.dma_start(wk_sb[:], wk[:].rearrange("(ko p) m -> p ko m", p=P))
            wv_sb = persist.tile([P, KO, DH], BF)
            nc.sync.dma_start(wv_sb[:], wv[:].rearrange("(ko p) m -> p ko m", p=P))
            wp_sb = persist.tile([P, 2, D], BF)
            nc.sync.dma_start(wp_sb[:], wp[:].rearrange("(j p) n -> p j n", p=P))
            ident = consts.tile([P, P], BF)
            make_identity(nc, ident[:])

            # ---- PE warm-up burst during the input-DMA window ----
            # The HAM clock gate keeps the PE at 1.2 GHz until it sees ~3.4us
            # of sustained matmul activity; without this burst every matmul in
            # the kernel runs at half clock. Junk matmuls on the identity tile
            # keep the PE busy from ~7us (identity ready) until the input DMAs
            # land, so the real matmuls start at 2.4 GHz and stay there.
            psw = ps_lg.tile([P, P], FP, tag="lg", name="warm")
            for _ in range(100):
                nc.tensor.matmul(psw[:], ident[:], ident[:], start=True, stop=True)

            # ---- projections ----
            # qT: [dh-part (2x128), seq]. kT: zero-padded per-head layout
            # [128, head, seq] with rows 64-127 = 0, so every logits matmul
            # contracts over K=128 (enables fast weight load); the junk rows
            # of the other operand multiply zeros.
            qT_sb = persist.tile([P, 2, NQ], BF)
            kT_pad = persist.tile([P, HPC, NK], BF)
            nc.gpsimd.memset(kT_pad[:], 0.0)
            for j in range(2):
                for qc in range(2):
                    ps = ps_sm.tile([P, 512], FP, tag="sm")
                    for ko in range(KO):
                        nc.tensor.matmul(
                            ps[:],
                            wq_sb[:, ko, j * P:(j + 1) * P],
                            xqT_sb[:, ko, qc * 512:(qc + 1) * 512],
                            start=(ko == 0),
                            stop=(ko == KO - 1),
                        )
                    nc.vector.tensor_copy(qT_sb[:, j, qc * 512:(qc + 1) * 512], ps[:])
            # Head h lands on partition rows (h%2)*64..+64 (matching where its
            # qT rows live); the other 64 rows stay zero. Odd heads use
            # column-group tiling so the matmul writes partitions 64-127.
            for h in range(HPC):
                p0 = (h % 2) * 64
                for qc in range(2):
                    ps = ps_sm.tile([P, 512], FP, tag="sm")
                    for ko in range(KO):
                        nc.tensor.matmul(
                            ps[p0:p0 + 64],
                            wk_sb[:, ko, h * HD:(h + 1) * HD],
                            xT_sb[:, ko, qc * 512:(qc + 1) * 512],
                            start=(ko == 0),
                            stop=(ko == KO - 1),
                        )
                    nc.vector.tensor_copy(
                        kT_pad[p0:p0 + 64, h, qc * 512:(qc + 1) * 512],
                        ps[p0:p0 + 64])

            # ---- v in natural layout [k-part, kj, head, hd], bf16 ----
            v_sb = persist.tile([P, 8, HPC, HD], BF)
            for kj in range(8):
                ps = ps_sm.tile([P, DH], FP, tag="sm")
                for ko in range(KO):
                    nc.tensor.matmul(
                        ps[:],
                        xT_sb[:, ko, kj * P:(kj + 1) * P],
                        wv_sb[:, ko, :],
                        start=(ko == 0),
                        stop=(ko == KO - 1),
                    )
                nc.vector.tensor_copy(
                    v_sb[:, kj].rearrange("p h d -> p (h d)"), ps[:]
                )

            # ---- per-head-pair attention ----
            # Heads 2*hp (partitions 0-63) and 2*hp+1 (partitions 64-127) run
            # concurrently in the PE via row-group tiling (tile_position is
            # auto-derived from the lhsT/rhs base partition).
            sums = consts.tile([P, HPC * 8], FP)   # row sums per (head, q-tile)
            rec = consts.tile([P, HPC * 8], FP)    # reciprocals
            out_n = persist.tile([P, 8, DH], BF)   # normalized attn @ v, [q, dh]
            outT = persist.tile([P, 2, NQ], BF)

            for hp in range(2):
                h0, h1 = 2 * hp, 2 * hp + 1
                for h in (h0, h1):
                    qT_f = qT_sb[:, hp]                # [128, NQ] (pair rows)
                    kT_h = kT_pad[:, h]                # [128, NK] (rows 64+ = 0)

                    # --- A+B interleaved: two independent PE->ACT streams ---
                    # A: logits [q, k] -> exp f32 (+ row sums) -> norm -> DMA
                    # B: logitsT [k, q] -> expT bf16 (feeds AV)
                    exa_tiles = {}
                    expT = expT_pool.tile([P, 8, NQ], BF, tag="expT")
                    for i in range(8):
                        psa = ps_lg.tile([P, NK], FP, tag="lg", name="psa")
                        for kc in range(2):
                            nc.tensor.matmul(
                                psa[:, kc * 512:(kc + 1) * 512],
                                qT_f[:, i * P:(i + 1) * P],
                                kT_h[:, kc * 512:(kc + 1) * 512],
                                start=True,
                                stop=True,
                            )
                        exa = exA_pool.tile([P, NK], FP, tag="exA")
                        si = h * 8 + i
                        nc.scalar.activation(
                            exa[:], psa[:], Exp, scale=SCALE,
                            accum_out=sums[:, si:si + 1],
                        )
                        exa_tiles[i] = exa

                        psb = ps_lg.tile([P, NQ], FP, tag="lg", name="psb")
                        for qc in range(2):
                            nc.tensor.matmul(
                                psb[:, qc * 512:(qc + 1) * 512],
                                kT_h[:, i * P:(i + 1) * P],
                                qT_f[:, qc * 512:(qc + 1) * 512],
                                start=True,
                                stop=True,
                            )
                        nc.scalar.activation(expT[:, i], psb[:], Exp, scale=SCALE)

                        if i in (3, 7):  # recip + normalize in half batches
                            lo, hi = (0, 4) if i == 3 else (4, 8)
                            nc.vector.reciprocal(
                                rec[:, h * 8 + lo:h * 8 + hi],
                                sums[:, h * 8 + lo:h * 8 + hi],
                            )
                            for mj in range(lo, hi):
                                exa = exa_tiles.pop(mj)
                                sj = h * 8 + mj
                                nc.vector.tensor_scalar_mul(
                                    exa[:], exa[:], rec[:, sj:sj + 1]
                                )
                                nc.sync.dma_start(
                                    attn_o[h, mj * P:(mj + 1) * P, :], exa[:]
                                )

                    # --- AV: out_u[q,64] = sum_kj expT_kj^T v_kj; scale ---
                    # All 8 q-tiles of this head accumulate into one PSUM
                    # bank; one broadcast-multiply normalizes and evacuates.
                    psv = ps_sm.tile([P, 8, HD], FP, tag="sm")
                    for mi in range(8):
                        for kj in range(8):
                            nc.tensor.matmul(
                                psv[:, mi],
                                expT[:, kj, mi * P:(mi + 1) * P],
                                v_sb[:, kj, h],
                                start=(kj == 0),
                                stop=(kj == 7),
                            )
                    nc.vector.tensor_tensor(
                        out_n[:, :, h * HD:(h + 1) * HD],
                        psv[:],
                        rec[:, h * 8:(h + 1) * 8].unsqueeze(-1).to_broadcast(
                            [P, 8, HD]),
                        mybir.AluOpType.mult,
                    )

                # --- transpose this pair's dh block: [q, 128] -> [128, q] ---
                for mi in range(8):
                    pst = ps_sm.tile([P, P], BF, tag="sm")
                    nc.tensor.transpose(
                        pst[:], out_n[:, mi, hp * P:(hp + 1) * P], ident[:]
                    )
                    nc.vector.tensor_copy(outT[:, hp, mi * P:(mi + 1) * P], pst[:])

                # --- this pair's half of the output projection ---
                # Host adds the two partials (along with the other core's),
                # so the pair-0 projection runs in the shadow of pair 1.
                for mi in range(8):
                    ps = ps_sm.tile([P, D], FP, tag="sm")
                    nc.tensor.matmul(
                        ps[:],
                        outT[:, hp, mi * P:(mi + 1) * P],
                        wp_sb[:, hp, :],
                        start=True,
                        stop=True,
                    )
                    of = outp.tile([P, D], FP, tag="of")
                    nc.vector.tensor_copy(of[:], ps[:])
                    nc.sync.dma_start(out_o[hp][mi * P:(mi + 1) * P, :], of[:])

    nc.compile()
    return nc


def get_nc():
    if "nc" not in _NC_CACHE:
        _NC_CACHE["nc"] = build_nc()
    return _NC_CACHE["nc"]


def make_in_maps(x, x_q, w_q, w_kv):
    """Shard full inputs into 8 per-core input maps (host-side numpy)."""
    import ml_dtypes

    bf = ml_dtypes.bfloat16
    x = np.asarray(x, dtype=np.float32)
    x_q = np.asarray(x_q, dtype=np.float32)
    w_q = np.asarray(w_q, dtype=np.float32)
    w_kv = np.asarray(w_kv, dtype=np.float32)
    xqT_b = [np.ascontiguousarray(x_q[b].T.astype(bf)) for b in range(B)]
    xT_b = [np.ascontiguousarray(x[b].T.astype(bf)) for b in range(B)]
    in_maps = []
    for c in range(NCORES):
        b, hg = c // 2, c % 2
        sl = slice(hg * DH, (hg + 1) * DH)
        in_maps.append({
            "xqT": xqT_b[b],
            "xT": xT_b[b],
            "wq": np.ascontiguousarray(w_q[:, sl].astype(bf)),
            "wk": np.ascontiguousarray(w_kv[:, sl].astype(bf)),
            "wv": np.ascontiguousarray(
                w_kv[:, D + hg * DH:D + (hg + 1) * DH].astype(bf)),
        })
    return in_maps


def make_in_maps_full(x, x_q, w_q, w_kv, w_proj):
    import ml_dtypes

    w_proj = np.asarray(w_proj, dtype=np.float32)
    in_maps = make_in_maps(x, x_q, w_q, w_kv)
    for c in range(NCORES):
        hg = c % 2
        sl = slice(hg * DH, (hg + 1) * DH)
        in_maps[c]["wp"] = np.ascontiguousarray(
            w_proj[sl, :].astype(ml_dtypes.bfloat16)
        )
    return in_maps


def unshard(results, b_proj):
    b_proj = np.asarray(b_proj, dtype=np.float32)
    attn = np.empty((B, H, NQ, NK), dtype=np.float32)
    out = np.empty((B, NQ, D), dtype=np.float32)
    for c in range(NCORES):
        b, hg = c // 2, c % 2
        attn[b, hg * HPC:(hg + 1) * HPC] = results[c]["attn_o"]
    for b in range(B):
        out[b] = (
            results[2 * b]["out_o0"] + results[2 * b]["out_o1"]
            + results[2 * b + 1]["out_o0"] + results[2 * b + 1]["out_o1"]
            + b_proj[None, :]
        )
    return out, attn


def kernel(x, x_q, w_q, w_kv, w_proj, b_proj):
    from concourse.bass_utils import run_bass_kernel_spmd

    nc = get_nc()
    in_maps = make_in_maps_full(x, x_q, w_q, w_kv, w_proj)
    res = run_bass_kernel_spmd(nc, in_maps, list(range(NCORES))).results
    return unshard(res, b_proj)


# revision 34
# speedup vs baseline: 1.2468x; 1.0016x over previous
"""Trainium2 Bass kernel for nn_Attention (B=4, Nq=Nk=1024, D=512, H=8).

Sharding: 8 cores = 4 batches x 2 head-groups (4 heads each).
Core c handles batch b = c // 2, heads [hg*4, hg*4+4) with hg = c % 2.

Per-core device program (all heavy math on device):
  qT = (x_q[b] @ w_q[:, hg])^T          [256, 1024]   (f32r matmuls)
  kT = (x[b]  @ w_k[:, hg])^T           [256, 1024]
  v  =  x[b]  @ w_v[:, hg]              [1024, 256]   (stored bf16)
  per head h (4):
    A: logits[q,k] = qT_h^T kT_h; exp (ACT, scale=0.125 fused, accum row
       sums); normalize in-place (gpsimd, per-partition recip); DMA -> attn
    B: logitsT[k,q] = kT_h^T qT_h; expT = exp (ACT, bf16)
    AV: out_u[q,64] = sum_kj expT_kj^T v_kj (bf16 matmuls);
        scale by recip (DVE, fused into PSUM->SBUF copy)
  transpose out [1024,256] -> outT (PE transposes), proj = outT^T @ w_p
  partial out -> DRAM (host adds the two head-group partials + bias).

Host side: slice/transpose inputs (numpy), run SPMD on 8 cores, concat
attn shards, add the two per-batch partials + b_proj.
"""

import sys

import numpy as np

for _p in ("/opt/trn_rl_repo",):
    if _p not in sys.path:
        sys.path.insert(0, _p)

# Problem constants (hardcoded per contest rules).
B, NQ, NK = 4, 1024, 1024
D = 512          # DIM_Q = DIM_K = OUT_DIM
H = 8
HD = 64          # head dim
SCALE = HD ** -0.5
HPC = 4          # heads per core
DH = HPC * HD    # 256: per-core slice of head dims
P = 128
KO = D // P      # 4 contraction chunks for the input projections
NCORES = 8

_NC_CACHE = {}


def build_nc():
    """Build the (single) SPMD Bass program. Same program on all 8 cores."""
    from contextlib import ExitStack

    import concourse.mybir as mybir
    import concourse.tile as tile
    from concourse import bacc
    from concourse.masks import make_identity

    FP = mybir.dt.float32
    BF = mybir.dt.bfloat16
    FR = mybir.dt.float32r
    Exp = mybir.ActivationFunctionType.Exp

    nc = bacc.Bacc("TRN2")
    xqT = nc.declare_dram_parameter("xqT", [D, NQ], BF, isOutput=False)
    xT = nc.declare_dram_parameter("xT", [D, NK], BF, isOutput=False)
    wq = nc.declare_dram_parameter("wq", [D, DH], BF, isOutput=False)
    wk = nc.declare_dram_parameter("wk", [D, DH], BF, isOutput=False)
    wv = nc.declare_dram_parameter("wv", [D, DH], BF, isOutput=False)
    wp = nc.declare_dram_parameter("wp", [DH, D], BF, isOutput=False)
    attn_o = nc.declare_dram_parameter("attn_o", [HPC, NQ, NK], FP, isOutput=True)
    out_o = [
        nc.declare_dram_parameter(f"out_o{j}", [NQ, D], FP, isOutput=True)
        for j in range(2)
    ]

    with tile.TileContext(nc) as tc:
        with ExitStack() as ctx:
            consts = ctx.enter_context(tc.tile_pool(name="consts", bufs=1))
            persist = ctx.enter_context(tc.tile_pool(name="persist", bufs=1))
            expT_pool = ctx.enter_context(tc.tile_pool(name="expT", bufs=3))
            exA_pool = ctx.enter_context(tc.tile_pool(name="exA", bufs=12))
            outp = ctx.enter_context(tc.tile_pool(name="outp", bufs=2))
            # PSUM budget (8 banks): lg 3x[128,1024]f32 = 6, sm 2x[128,512]f32 = 2
            ps_sm = ctx.enter_context(tc.tile_pool(name="ps_sm", bufs=2, space="PSUM"))
            ps_lg = ctx.enter_context(tc.tile_pool(name="ps_lg", bufs=3, space="PSUM"))

            # ---- load inputs (split so compute starts early, spread queues) ----
            xqT_sb = persist.tile([P, KO, NQ], BF)
            xT_sb = persist.tile([P, KO, NK], BF)
            for ko in range(KO):
                for sh in range(2):
                    nc.sync.dma_start(
                        xqT_sb[:, ko, sh * 512:(sh + 1) * 512],
                        xqT[:].rearrange("(ko p) q -> ko p q", p=P)
                        [ko, :, sh * 512:(sh + 1) * 512])
                    nc.sync.dma_start(
                        xT_sb[:, ko, sh * 512:(sh + 1) * 512],
                        xT[:].rearrange("(ko p) q -> ko p q", p=P)
                        [ko, :, sh * 512:(sh + 1) * 512])
            wq_sb = persist.tile([P, KO, DH], BF)
            nc.sync.dma_start(wq_sb[:], wq[:].rearrange("(ko p) m -> p ko m", p=P))
            wk_sb = persist.tile([P, KO, DH], BF)
            nc.sync.dma_start(wk_sb[:], wk[:].rearrange("(ko p) m -> p ko m", p=P))
            wv_sb = persist.tile([P, KO, DH], BF)
            nc.sync.dma_start(wv_sb[:], wv[:].rearrange("(ko p) m -> p ko m", p=P))
            wp_sb = persist.tile([P, 2, D], BF)
            nc.sync.dma_start(wp_sb[:], wp[:].rearrange("(j p) n -> p j n", p=P))
            ident = consts.tile([P, P], BF)
            make_identity(nc, ident[:])

            # ---- PE warm-up burst during the input-DMA window ----
            # The HAM clock gate keeps the PE at 1.2 GHz until it sees ~3.4us
            # of sustained matmul activity; without this burst every matmul in
            # the kernel runs at half clock. Junk matmuls on the identity tile
            # keep the PE busy from ~7us (identity ready) until the input DMAs
            # land, so the real matmuls start at 2.4 GHz and stay there.
            psw = ps_lg.tile([P, P], FP, tag="lg", name="warm")
            for _ in range(100):
                nc.tensor.matmul(psw[:], ident[:], ident[:], start=True, stop=True)

            # ---- projections ----
            # qT: [dh-part (2x128), seq]. kT: zero-padded per-head layout
            # [128, head, seq] with rows 64-127 = 0, so every logits matmul
            # contracts over K=128 (enables fast weight load); the junk rows
            # of the other operand multiply zeros.
            qT_sb = persist.tile([P, 2, NQ], BF)
            kT_pad = persist.tile([P, HPC, NK], BF)
            nc.gpsimd.memset(kT_pad[:], 0.0)
            def emit_qT(j):
                for qc in range(2):
                    ps = ps_sm.tile([P, 512], FP, tag="sm", name="psq")
                    for ko in range(KO):
                        nc.tensor.matmul(
                            ps[:],
                            wq_sb[:, ko, j * P:(j + 1) * P],
                            xqT_sb[:, ko, qc * 512:(qc + 1) * 512],
                            start=(ko == 0),
                            stop=(ko == KO - 1),
                        )
                    nc.vector.tensor_copy(qT_sb[:, j, qc * 512:(qc + 1) * 512], ps[:])

            # Head h lands on partition rows (h%2)*64..+64 (matching where its
            # qT rows live); the other 64 rows stay zero. Odd heads use
            # column-group tiling so the matmul writes partitions 64-127.
            def emit_kT(h):
                p0 = (h % 2) * 64
                for qc in range(2):
                    ps = ps_sm.tile([P, 512], FP, tag="sm", name="psk")
                    for ko in range(KO):
                        nc.tensor.matmul(
                            ps[p0:p0 + 64],
                            wk_sb[:, ko, h * HD:(h + 1) * HD],
                            xT_sb[:, ko, qc * 512:(qc + 1) * 512],
                            start=(ko == 0),
                            stop=(ko == KO - 1),
                        )
                    nc.vector.tensor_copy(
                        kT_pad[p0:p0 + 64, h, qc * 512:(qc + 1) * 512],
                        ps[p0:p0 + 64])

            emit_qT(0)
            emit_kT(0)
            emit_kT(1)
            emit_qT(1)
            emit_kT(2)
            emit_kT(3)

            # ---- v in natural layout [k-part, kj, head, hd], bf16 ----
            v_sb = persist.tile([P, 8, HPC, HD], BF)
            for kj in range(8):
                ps = ps_sm.tile([P, DH], FP, tag="sm")
                for ko in range(KO):
                    nc.tensor.matmul(
                        ps[:],
                        xT_sb[:, ko, kj * P:(kj + 1) * P],
                        wv_sb[:, ko, :],
                        start=(ko == 0),
                        stop=(ko == KO - 1),
                    )
                nc.vector.tensor_copy(
                    v_sb[:, kj].rearrange("p h d -> p (h d)"), ps[:]
                )

            # ---- per-head-pair attention ----
            # Heads 2*hp (partitions 0-63) and 2*hp+1 (partitions 64-127) run
            # concurrently in the PE via row-group tiling (tile_position is
            # auto-derived from the lhsT/rhs base partition).
            sums = consts.tile([P, HPC * 8], FP)   # row sums per (head, q-tile)
            rec = consts.tile([P, HPC * 8], FP)    # reciprocals
            out_n = persist.tile([P, 8, DH], BF)   # normalized attn @ v, [q, dh]
            outT = persist.tile([P, 2, NQ], BF)

            for hp in range(2):
                h0, h1 = 2 * hp, 2 * hp + 1
                for h in (h0, h1):
                    qT_f = qT_sb[:, hp]                # [128, NQ] (pair rows)
                    kT_h = kT_pad[:, h]                # [128, NK] (rows 64+ = 0)

                    # --- A+B interleaved: two independent PE->ACT streams ---
                    # A: logits [q, k] -> exp f32 (+ row sums) -> norm -> DMA
                    # B: logitsT [k, q] -> expT bf16 (feeds AV)
                    exa_tiles = {}
                    expT = expT_pool.tile([P, 8, NQ], BF, tag="expT")
                    for i in range(8):
                        psa = ps_lg.tile([P, NK], FP, tag="lg", name="psa")
                        for kc in range(2):
                            nc.tensor.matmul(
                                psa[:, kc * 512:(kc + 1) * 512],
                                qT_f[:, i * P:(i + 1) * P],
                                kT_h[:, kc * 512:(kc + 1) * 512],
                                start=True,
                                stop=True,
                            )
                        exa = exA_pool.tile([P, NK], FP, tag="exA")
                        si = h * 8 + i
                        nc.scalar.activation(
                            exa[:], psa[:], Exp, scale=SCALE,
                            accum_out=sums[:, si:si + 1],
                        )
                        exa_tiles[i] = exa

                        psb = ps_lg.tile([P, NQ], FP, tag="lg", name="psb")
                        for qc in range(2):
                            nc.tensor.matmul(
                                psb[:, qc * 512:(qc + 1) * 512],
                                kT_h[:, i * P:(i + 1) * P],
                                qT_f[:, qc * 512:(qc + 1) * 512],
                                start=True,
                                stop=True,
                            )
                        nc.scalar.activation(expT[:, i], psb[:], Exp, scale=SCALE)

                        if i in (3, 7):  # recip + normalize in half batches
                            lo, hi = (0, 4) if i == 3 else (4, 8)
                            nc.vector.reciprocal(
                                rec[:, h * 8 + lo:h * 8 + hi],
                                sums[:, h * 8 + lo:h * 8 + hi],
                            )
                            for mj in range(lo, hi):
                                exa = exa_tiles.pop(mj)
                                sj = h * 8 + mj
                                nc.vector.tensor_scalar_mul(
                                    exa[:], exa[:], rec[:, sj:sj + 1]
                                )
                                nc.sync.dma_start(
                                    attn_o[h, mj * P:(mj + 1) * P, :], exa[:]
                                )

                    # --- AV: out_u[q,64] = sum_kj expT_kj^T v_kj; scale ---
                    # All 8 q-tiles of this head accumulate into one PSUM
                    # bank; one broadcast-multiply normalizes and evacuates.
                    psv = ps_sm.tile([P, 8, HD], FP, tag="sm")
                    for mi in range(8):
                        for kj in range(8):
                            nc.tensor.matmul(
                                psv[:, mi],
                                expT[:, kj, mi * P:(mi + 1) * P],
                                v_sb[:, kj, h],
                                start=(kj == 0),
                                stop=(kj == 7),
                            )
                    nc.vector.tensor_tensor(
                        out_n[:, :, h * HD:(h + 1) * HD],
                        psv[:],
                        rec[:, h * 8:(h + 1) * 8].unsqueeze(-1).to_broadcast(
                            [P, 8, HD]),
                        mybir.AluOpType.mult,
                    )

                # --- transpose this pair's dh block: [q, 128] -> [128, q] ---
                for mi in range(8):
                    pst = ps_sm.tile([P, P], BF, tag="sm")
                    nc.tensor.transpose(
                        pst[:], out_n[:, mi, hp * P:(hp + 1) * P], ident[:]
                    )
                    nc.vector.tensor_copy(outT[:, hp, mi * P:(mi + 1) * P], pst[:])

                # --- this pair's half of the output projection ---
                # Host adds the two partials (along with the other core's),
                # so the pair-0 projection runs in the shadow of pair 1.
                for mi in range(8):
                    ps = ps_sm.tile([P, D], FP, tag="sm")
                    nc.tensor.matmul(
                        ps[:],
                        outT[:, hp, mi * P:(mi + 1) * P],
                        wp_sb[:, hp, :],
                        start=True,
                        stop=True,
                    )
                    of = outp.tile([P, D], FP, tag="of")
                    nc.vector.tensor_copy(of[:], ps[:])
                    nc.sync.dma_start(out_o[hp][mi * P:(mi + 1) * P, :], of[:])

    nc.compile()
    return nc


def get_nc():
    if "nc" not in _NC_CACHE:
        _NC_CACHE["nc"] = build_nc()
    return _NC_CACHE["nc"]


def make_in_maps(x, x_q, w_q, w_kv):
    """Shard full inputs into 8 per-core input maps (host-side numpy)."""
    import ml_dtypes

    bf = ml_dtypes.bfloat16
    x = np.asarray(x, dtype=np.float32)
    x_q = np.asarray(x_q, dtype=np.float32)
    w_q = np.asarray(w_q, dtype=np.float32)
    w_kv = np.asarray(w_kv, dtype=np.float32)
    xqT_b = [np.ascontiguousarray(x_q[b].T.astype(bf)) for b in range(B)]
    xT_b = [np.ascontiguousarray(x[b].T.astype(bf)) for b in range(B)]
    in_maps = []
    for c in range(NCORES):
        b, hg = c // 2, c % 2
        sl = slice(hg * DH, (hg + 1) * DH)
        in_maps.append({
            "xqT": xqT_b[b],
            "xT": xT_b[b],
            "wq": np.ascontiguousarray(w_q[:, sl].astype(bf)),
            "wk": np.ascontiguousarray(w_kv[:, sl].astype(bf)),
            "wv": np.ascontiguousarray(
                w_kv[:, D + hg * DH:D + (hg + 1) * DH].astype(bf)),
        })
    return in_maps


def make_in_maps_full(x, x_q, w_q, w_kv, w_proj):
    import ml_dtypes

    w_proj = np.asarray(w_proj, dtype=np.float32)
    in_maps = make_in_maps(x, x_q, w_q, w_kv)
    for c in range(NCORES):
        hg = c % 2
        sl = slice(hg * DH, (hg + 1) * DH)
        in_maps[c]["wp"] = np.ascontiguousarray(
            w_proj[sl, :].astype(ml_dtypes.bfloat16)
        )
    return in_maps


def unshard(results, b_proj):
    b_proj = np.asarray(b_proj, dtype=np.float32)
    attn = np.empty((B, H, NQ, NK), dtype=np.float32)
    out = np.empty((B, NQ, D), dtype=np.float32)
    for c in range(NCORES):
        b, hg = c // 2, c % 2
        attn[b, hg * HPC:(hg + 1) * HPC] = results[c]["attn_o"]
    for b in range(B):
        out[b] = (
            results[2 * b]["out_o0"] + results[2 * b]["out_o1"]
            + results[2 * b + 1]["out_o0"] + results[2 * b + 1]["out_o1"]
            + b_proj[None, :]
        )
    return out, attn


def kernel(x, x_q, w_q, w_kv, w_proj, b_proj):
    from concourse.bass_utils import run_bass_kernel_spmd

    nc = get_nc()
    in_maps = make_in_maps_full(x, x_q, w_q, w_kv, w_proj)
    res = run_bass_kernel_spmd(nc, in_maps, list(range(NCORES))).results
    return unshard(res, b_proj)
